# revision 96
# baseline (speedup 1.0000x reference)
"""Distributed FFT (N = 2^24 complex points) on 8 Trainium2 NeuronCores.

Four-step (Cooley-Tukey) decomposition N = 4096 x 4096:
  launch 1: per global column j1g, FFT_4096 over j2g      (batch parallel over j1g)
  host:     global twiddle wN^{j1g*k2g} + transpose exchange
  launch 2: per global row k2g, FFT_4096 over j1g         (batch parallel over k2g)

Both launches run the SAME compiled SPMD kernel on all 8 cores: a batch of
512 local FFT_4096 per core. Each FFT_4096 = radix-32 stage (block-diag 4x
packed over the contraction axis, K=128) fused with its inter-stage transpose
(data-stationary matmul: psum[j1, :] += S_slice.T @ [[Wr|Wi],[-Wi|Wr]]),
then a radix-128 stage whose twiddle exp(-2pi i j1 kap2/4096) is folded into
32 per-kap2 weight matrices {Br, Bi, -Bi}.

All wire traffic (inputs, weights, outputs) is bfloat16: the kernel is
DMA-bound and bf16 halves HBM bytes while the PE runs bf16 matmuls at the
same 1 column/cycle as f32r. Host-side marshalling lays every DRAM tensor
out so each DMA moves >=2KB contiguous runs per partition (large
descriptors). PSUM (fp32) evacuations are load-balanced across DVE and
Activation (GPSIMD cannot access PSUM). Stage order A0,A1,A2,B0,A3,B1
relaxes every DMA deadline; warmup matmuls pre-ramp the PE p-state; the
first kps after the A3|B1 boundary split their matmuls by c2-half so the
PE bridges the final evacuation latency.

Local FFT_4096 digits: f = j1 + 128*j2 (j1 in [0,128) fast, j2 in [0,32));
k = kap2 + 32*kap1. Batch b = 128*t + 32*g + s (t chunk of 128, g K-pack
group, s in [0,32)). Host does all layout marshalling (numpy index shuffles);
device sees only contiguous [128, X] DMAs.
"""
import numpy as np
import ml_dtypes

import concourse.mybir as mybir
import concourse.tile as tile
from concourse import bacc
from concourse.bass_utils import run_bass_kernel_spmd

NG = 4096                 # global matrix dimension; N = NG*NG
N = NG * NG
NCORES = 8
BPC = NG // NCORES        # 512 signals per core per launch
NCHUNK = 4                # chunks of 128 signals
import os as _os
N_WARMUP = int(_os.environ.get("N_WARMUP", "8"))

_F32 = mybir.dt.float32
_BF16 = mybir.dt.bfloat16
_NPBF16 = ml_dtypes.bfloat16

# ---------------------------------------------------------------------------
# constants (host-side numpy)
# ---------------------------------------------------------------------------

_consts_cache = None


def _make_consts():
    """bdc: [128, 512] bf16  (p = 32g+j2; cols = 256*srcpl + 128*pl' + 32g+kap2)
    bm:  [128, 32, 3, 128] bf16 (p = j1; kap2, {Br,Bi,-Bi}, kap1)"""
    global _consts_cache
    if _consts_cache is not None:
        return _consts_cache
    j2 = np.arange(32)
    W32 = np.exp(-2j * np.pi * np.outer(j2, j2) / 32)
    I4 = np.eye(4)
    BDr = np.kron(I4, W32.real)
    BDi = np.kron(I4, W32.imag)
    bdc = np.concatenate([
        np.concatenate([BDr, BDi], axis=1),     # applied to Sr
        np.concatenate([-BDi, BDr], axis=1),    # applied to Si
    ], axis=1).astype(_NPBF16)                  # [128, 512]

    j1 = np.arange(128)
    W128 = np.exp(-2j * np.pi * np.outer(j1, j1) / 128)
    bm = np.zeros((128, 32, 3, 128), np.float32)
    for kap2 in range(32):
        B = np.exp(-2j * np.pi * j1 * kap2 / 4096)[:, None] * W128  # [j1][kap1]
        bm[:, kap2, 0] = B.real
        bm[:, kap2, 1] = B.imag
        bm[:, kap2, 2] = -B.imag
    bm = bm.astype(_NPBF16)
    _consts_cache = (np.ascontiguousarray(bdc), np.ascontiguousarray(bm))
    return _consts_cache


_tw_cache = None


def _global_twiddle():
    """exp(-2pi i k2g*j1g / N) as complex64 [NG, NG] (k2g rows)."""
    global _tw_cache
    if _tw_cache is None:
        k = np.arange(NG, dtype=np.float64)
        phase = np.outer(k, k) * (-2.0 * np.pi / N)
        _tw_cache = np.exp(1j * phase).astype(np.complex64)
    return _tw_cache


# ---------------------------------------------------------------------------
# marshalling (host)
# ---------------------------------------------------------------------------

def _marshal_in(Vre, Vim):
    """Vre/Vim: [4096 f][512 b] f32 planes -> in2 [4,4,128,2,8,128] bf16
    (t, q, p=32g+j2, pl, s8, j1); s = 8q + s8."""
    out = np.empty((NCHUNK, 4, 128, 2, 8, 128), _NPBF16)
    for pl, V in ((0, Vre), (1, Vim)):
        A = V.reshape(32, 128, 4, 4, 4, 8)     # j2, j1, t, g, q, s8
        out[:, :, :, pl] = (
            A.transpose(2, 4, 3, 0, 5, 1)       # t, q, g, j2, s8, j1
            .reshape(NCHUNK, 4, 128, 8, 128)
            .astype(_NPBF16)
        )
    return out


def _unmarshal_out(O):
    """out2 [2,16,128,2,512] bf16 (sc, kp, kap1, pl, n2; n2 = 256u+128c2+4s+g)
    -> (Fre, Fim) planes [4096 k][512 b] f32."""
    O8 = O.reshape(2, 16, 128, 2, 2, 2, 32, 4)  # sc, kp, kap1, pl, u, c2, s, g
    # kap2 = 2*kp+u ; k = 32*kap1 + kap2 ; b = 256*sc + 128*c2 + 32*g + s
    P = np.ascontiguousarray(
        O8.transpose(3, 2, 1, 4, 0, 5, 7, 6)    # pl, kap1, kp, u, sc, c2, g, s
    ).reshape(2, NG, BPC).astype(np.float32)
    return P[0], P[1]


# ---------------------------------------------------------------------------
# device kernel (Bass/Tile), shared by both launches
# ---------------------------------------------------------------------------

_nc_cache = None


def _build_nc():
    global _nc_cache
    if _nc_cache is not None:
        return _nc_cache

    nc = bacc.Bacc(trn_type="TRN2")
    # in layout: [t, q, p = 32g+j2, (pl, s8, j1)]
    in_d = nc.dram_tensor("in2", [NCHUNK, 4, 128, 2, 8, 128], _BF16,
                          kind="ExternalInput")
    bdc_d = nc.dram_tensor("bdc", [128, 512], _BF16, kind="ExternalInput")
    bm_d = nc.dram_tensor("bm", [128, 32, 3, 128], _BF16, kind="ExternalInput")
    # out layout: [superchunk, kap2pair, kap1, pl, n2], n2 = 256u+128c2+4s+g
    out_d = nc.dram_tensor("out2", [NCHUNK // 2, 16, 128, 2, 512], _BF16,
                           kind="ExternalOutput")

    with tile.TileContext(nc) as tc:
        with (
            tc.tile_pool(name="consts", bufs=1) as cpool,
            tc.tile_pool(name="inp", bufs=6) as inpool,
            tc.tile_pool(name="inh", bufs=8) as inhpool,
            tc.tile_pool(name="tp", bufs=2) as tpool,
            tc.tile_pool(name="outp", bufs=16) as outpool,
            tc.tile_pool(name="scr", bufs=4) as scrpool,
            tc.tile_pool(name="pA", bufs=4, space="PSUM") as pA,
            tc.tile_pool(name="pB", bufs=4, space="PSUM") as pB,
        ):
            # --- greedy balancing of PSUM evacuations over DVE + ACT -------
            # (GPSIMD/Pool cannot access PSUM on TRN2 — BIR verifier)
            eng_load = {"v": 0.0, "s": 0.0}
            # rough ns per [128, cols] op: per-elem rate + fixed overhead
            eng_cost = {
                "v": lambda cols: cols * 1.042 + 190.0,   # DVE @0.96GHz
                "s": lambda cols: cols * 0.833 + 230.0,   # ACT @1.2GHz
            }

            def _pick(cols):
                e = min(eng_load, key=lambda k: eng_load[k] + eng_cost[k](cols))
                eng_load[e] += eng_cost[e](cols)
                return e

            def evac(out_ap, in_ap, cols, fast=False, allow=None):
                if allow:
                    e = min(allow, key=lambda k: eng_load[k] + eng_cost[k](cols))
                    eng_load[e] += eng_cost[e](cols)
                else:
                    e = _pick(cols)
                if e == "v":
                    nc.vector.tensor_copy(out_ap, in_ap)
                else:
                    nc.scalar.copy(out_ap, in_ap)

            # --- resident constants ---------------------------------------
            bdc_t = cpool.tile([128, 512], _BF16, tag="bdc")
            bm_t = cpool.tile([128, 32, 3, 128], _BF16, tag="bm")

            in_tiles = {}

            def load_input(t, q, split=1):
                """split=2 halves the DMA (earlier first matmul at startup)."""
                if split == 1:
                    it = inpool.tile([128, 2, 8, 128], _BF16, tag="in")
                    nc.sync.dma_start(it[:], in_d[t, q])
                    in_tiles[(t, q)] = [it]
                else:
                    tiles = []
                    for hh in range(2):
                        it = inhpool.tile([128, 2, 4, 128], _BF16, tag="inh")
                        nc.sync.dma_start(it[:], in_d[t, q, :, :, 4 * hh:4 * hh + 4])
                        tiles.append(it)
                    in_tiles[(t, q)] = tiles

            def slice_in(t, s):
                """stationary [128, 128] slices (re, im) for s within chunk t."""
                tiles = in_tiles[(t, s // 8)]
                r = s % 8
                it = tiles[0] if len(tiles) == 1 or r < 4 else tiles[1]
                r = r if len(tiles) == 1 else r % 4
                return it[:, 0, r], it[:, 1, r]

            # PE warmup: dependency-free matmuls on a memset tile keep the
            # tensor engine continuously busy from t~0.3us, so the p-state
            # ramp (half clock for the first 3us of busy) completes before
            # the first real matmul instead of slowing it down.
            wz = scrpool.tile([128, 512], _BF16, tag="warm")
            nc.vector.memset(wz[:], 0.0)
            wbank = pA.tile([128, 512], _F32, tag="psA")
            for _ in range(N_WARMUP):
                nc.tensor.matmul(wbank[:], wz[:, 0:128], wz[:],
                                 start=True, stop=True)

            # DMA order: bdc first (needed by the first matmul), then input
            # chunks 0-2; the three bm pieces ride between the chunk-2
            # quarters (the A0,A1,A2,B0,A3,B1 stage order gives them slack).
            nc.sync.dma_start(bdc_t[:], bdc_d.ap())
            for q in range(4):
                load_input(0, q, split=2)
            for q in range(4):
                load_input(1, q)
            for q in range(4):
                load_input(2, q)
                if q == 1:
                    nc.sync.dma_start(bm_t[:, 0:8], bm_d.ap()[:, 0:8])
            nc.sync.dma_start(bm_t[:, 8:16], bm_d.ap()[:, 8:16])
            nc.sync.dma_start(bm_t[:, 16:32], bm_d.ap()[:, 16:32])
            for q in range(4):
                load_input(3, q)

            tts = []
            for sc in range(NCHUNK // 2):
                tt = tpool.tile([128, 2, 32, 2, 4, 32], _BF16, tag="tt")
                tts.append((tt, tt.rearrange("p a b c d e -> p (a b c d e)")))

            def stage_a(t):
                # ---- fused stage A + transpose: per s:
                #   psum[j1, pl*128 + 32g+kap2] += S_sl.T @ bdc[pl-combo]
                tt, ttf = tts[t // 2]
                c2 = t % 2
                for sp in range(16):             # s-pairs within chunk
                    bank = pA.tile([128, 512], _F32, tag="psA")
                    for e in range(2):
                        sl = 2 * sp + e          # s within chunk
                        sre, sim = slice_in(t, sl)
                        ys = slice(256 * e, 256 * e + 256)
                        nc.tensor.matmul(bank[:, ys], sre,
                                         bdc_t[:, 0:256], start=True, stop=False)
                        nc.tensor.matmul(bank[:, ys], sim,
                                         bdc_t[:, 256:512], start=False, stop=True)
                    off = (c2 * 32 + 2 * sp) * 256
                    if t == 3 and sp >= 8:
                        # the last banks gate stage B (it reads all of tt):
                        # strict bank-level alternation keeps both queues
                        # short so the final evac lands promptly
                        if sp % 2 == 0:
                            nc.vector.tensor_copy(ttf[:, off:off + 512], bank[:])
                            eng_load["v"] += eng_cost["v"](512)
                        else:
                            nc.scalar.copy(ttf[:, off:off + 512], bank[:])
                            eng_load["s"] += eng_cost["s"](512)
                    else:
                        evac(ttf[:, off:off + 512], bank[:], 512)

            def stage_b(sc):
                # ---- stage B: radix-128, per-kap2 twiddled weights, N=256
                tt, ttf = tts[sc]
                for kp in range(16):             # kap2 pairs
                    tail = (sc == NCHUNK // 2 - 1) and kp == 15
                    kp_allow = None
                    if tail:
                        # progressive split of the final kp: each sub-group's
                        # evac+DMA drains while the next sub-group's matmuls
                        # run, so only a tiny store chain trails the last
                        # matmul
                        ot = outpool.tile([128, 2, 512], _BF16, tag="out")
                        for u in range(2):
                            kap2 = 2 * kp + u
                            us = slice(256 * u, 256 * u + 256)
                            yru = pA.tile([128, 256], _F32, tag="psA")
                            yiu = pA.tile([128, 256], _F32, tag="psA")
                            trs = tt[:, :, :, 0, :, kap2]
                            tis = tt[:, :, :, 1, :, kap2]
                            br = bm_t[:, kap2, 0]
                            bi = bm_t[:, kap2, 1]
                            bni = bm_t[:, kap2, 2]
                            nc.tensor.matmul(yru[:], br, trs, start=True, stop=False)
                            nc.tensor.matmul(yiu[:], br, tis, start=True, stop=False)
                            nc.tensor.matmul(yru[:], bni, tis, start=False, stop=True)
                            nc.tensor.matmul(yiu[:], bi, trs, start=False, stop=True)
                            nc.vector.tensor_copy(ot[:, 0, us], yru[:])
                            nc.scalar.copy(ot[:, 1, us], yiu[:])
                        nc.sync.dma_start(out_d[sc, kp], ot[:])
                        continue
                    # the first kps after the A3|B1 boundary borrow the idle
                    # pA banks: extra psum headroom while the boundary evac
                    # backlog drains
                    boundary = sc == 1 and kp < 2
                    pool = pA if boundary else pB
                    yr = pool.tile([128, 512], _F32, tag="psA" if pool is pA else "psB")
                    yi = pool.tile([128, 512], _F32, tag="psA" if pool is pA else "psB")
                    if boundary:
                        # c2-split matmul order: the c2=0 half of tt(sc1) was
                        # written by A2 (long done), so these matmuls bridge
                        # the bubble while A3's last evacuations land
                        for c2v in range(2):
                            for u in range(2):
                                kap2 = 2 * kp + u
                                ys = slice(256 * u + 128 * c2v,
                                           256 * u + 128 * c2v + 128)
                                trs = tt[:, c2v, :, 0, :, kap2]
                                tis = tt[:, c2v, :, 1, :, kap2]
                                br = bm_t[:, kap2, 0]
                                bi = bm_t[:, kap2, 1]
                                bni = bm_t[:, kap2, 2]
                                nc.tensor.matmul(yr[:, ys], br, trs,
                                                 start=True, stop=False)
                                nc.tensor.matmul(yi[:, ys], br, tis,
                                                 start=True, stop=False)
                                nc.tensor.matmul(yr[:, ys], bni, tis,
                                                 start=False, stop=True)
                                nc.tensor.matmul(yi[:, ys], bi, trs,
                                                 start=False, stop=True)
                    else:
                        for u in range(2):
                            kap2 = 2 * kp + u
                            ys = slice(256 * u, 256 * u + 256)
                            trs = tt[:, :, :, 0, :, kap2]
                            tis = tt[:, :, :, 1, :, kap2]
                            br = bm_t[:, kap2, 0]
                            bi = bm_t[:, kap2, 1]
                            bni = bm_t[:, kap2, 2]
                            nc.tensor.matmul(yr[:, ys], br, trs, start=True, stop=False)
                            nc.tensor.matmul(yi[:, ys], br, tis, start=True, stop=False)
                            nc.tensor.matmul(yr[:, ys], bni, tis, start=False, stop=True)
                            nc.tensor.matmul(yi[:, ys], bi, trs, start=False, stop=True)
                    # evac + store
                    ot = outpool.tile([128, 2, 512], _BF16, tag="out")
                    evac(ot[:, 0], yr[:], 512, allow=kp_allow)
                    evac(ot[:, 1], yi[:], 512, allow=kp_allow)
                    nc.sync.dma_start(out_d[sc, kp], ot[:])

            # A2 runs before B0: every weight/input DMA deadline gains the
            # width of one stage-A chunk, removing the early supply crunch
            stage_a(0)
            stage_a(1)
            stage_a(2)
            stage_b(0)
            stage_a(3)
            stage_b(1)

    nc.finalize()
    _nc_cache = nc
    return nc


# ---------------------------------------------------------------------------
# launch helper
# ---------------------------------------------------------------------------

_last_exec_ns = None


def last_exec_time_ns():
    """Sum of HW exec times (ns) of the launches in the last kernel() call,
    when KERNEL_TRACE=1 was set and NTFF profiling is available. None otherwise."""
    return _last_exec_ns


def predicted_exec_time_ns():
    """Cost-model (TimelineSim) predicted HW exec time for both launches, ns."""
    from concourse.timeline_sim import TimelineSim
    nc = _build_nc()
    return int(2 * TimelineSim(nc).simulate())


def _run_launch(cols_re, cols_im):
    """cols_re/cols_im: list of 8 planes [4096 f][512 b] f32.
    Returns list of 8 (Fre, Fim) planes [4096 k][512 b]."""
    global _last_exec_ns
    import os
    nc = _build_nc()
    bdc, bm = _make_consts()
    in_maps = []
    for c in range(NCORES):
        in_maps.append({
            "in2": _marshal_in(cols_re[c], cols_im[c]),
            "bdc": bdc, "bm": bm,
        })
    trace = bool(os.environ.get("KERNEL_TRACE"))
    try:
        res = run_bass_kernel_spmd(nc, in_maps, core_ids=list(range(NCORES)),
                                   trace=trace)
    except ModuleNotFoundError:
        # NTFF profiling hook unavailable under this axon client; run untraced.
        res = run_bass_kernel_spmd(nc, in_maps, core_ids=list(range(NCORES)))
    if trace and getattr(res, "exec_time_ns", None) is not None:
        _last_exec_ns = (_last_exec_ns or 0) + res.exec_time_ns
    return [_unmarshal_out(np.asarray(res.results[c]["out2"]))
            for c in range(NCORES)]


# ---------------------------------------------------------------------------
# public entry point
# ---------------------------------------------------------------------------

def kernel(x: np.ndarray) -> np.ndarray:
    """x: [N, 2] float32 (re, im). Returns FFT(x) as [N, 2] float32."""
    global _last_exec_ns
    _last_exec_ns = None
    x = np.asarray(x)
    Are = np.ascontiguousarray(x[:, 0].reshape(NG, NG))  # [j2g][j1g]
    Aim = np.ascontiguousarray(x[:, 1].reshape(NG, NG))

    # launch 1: FFT over rows (j2g) for each column j1g
    cols_re = [np.ascontiguousarray(Are[:, BPC * c:BPC * (c + 1)]) for c in range(NCORES)]
    cols_im = [np.ascontiguousarray(Aim[:, BPC * c:BPC * (c + 1)]) for c in range(NCORES)]
    l1 = _run_launch(cols_re, cols_im)

    # host: assemble F [k2g][j1g], twiddle, transpose-exchange
    F = np.empty((NG, NG), np.complex64)
    for c in range(NCORES):
        fre, fim = l1[c]
        F[:, BPC * c:BPC * (c + 1)] = fre + 1j * fim
    F *= _global_twiddle()

    # launch 2: FFT over j1g for each row k2g; core d gets rows [512d, 512(d+1))
    cols_re2 = []
    cols_im2 = []
    for d in range(NCORES):
        block = F[BPC * d:BPC * (d + 1), :].T      # [j1g][k2g-local]
        cols_re2.append(np.ascontiguousarray(block.real))
        cols_im2.append(np.ascontiguousarray(block.imag))
    l2 = _run_launch(cols_re2, cols_im2)

    # assemble Xmat [k1g][k2g]; out flat index k = 4096*k1g + k2g
    out = np.empty((NG, NG, 2), np.float32)
    for d in range(NCORES):
        rre, rim = l2[d]
        out[:, BPC * d:BPC * (d + 1), 0] = rre
        out[:, BPC * d:BPC * (d + 1), 1] = rim
    return out.reshape(N, 2)


# revision 100
# speedup vs baseline: 1.0148x; 1.0148x over previous
"""Distributed FFT (N = 2^24 complex points) on 8 Trainium2 NeuronCores.

Four-step (Cooley-Tukey) decomposition N = 4096 x 4096:
  launch 1: per global column j1g, FFT_4096 over j2g      (batch parallel over j1g)
  host:     global twiddle wN^{j1g*k2g} + transpose exchange
  launch 2: per global row k2g, FFT_4096 over j1g         (batch parallel over k2g)

Both launches run the SAME compiled SPMD kernel on all 8 cores: a batch of
512 local FFT_4096 per core. Each FFT_4096 = radix-32 stage (block-diag 4x
packed over the contraction axis, K=128) fused with its inter-stage transpose
(data-stationary matmul: psum[j1, :] += S_slice.T @ [[Wr|Wi],[-Wi|Wr]]),
then a radix-128 stage whose twiddle exp(-2pi i j1 kap2/4096) is folded into
32 per-kap2 weight matrices {Br, Bi, -Bi}.

All wire traffic (inputs, weights, outputs) is bfloat16: the kernel is
DMA-bound and bf16 halves HBM bytes while the PE runs bf16 matmuls at the
same 1 column/cycle as f32r. Host-side marshalling lays every DRAM tensor
out so each DMA moves >=2KB contiguous runs per partition (large
descriptors). PSUM (fp32) evacuations are load-balanced across DVE and
Activation (GPSIMD cannot access PSUM). Stage order A0,A1,A2,B0,A3,B1
relaxes every DMA deadline; warmup matmuls pre-ramp the PE p-state; the
first kps after the A3|B1 boundary split their matmuls by c2-half so the
PE bridges the final evacuation latency.

Local FFT_4096 digits: f = j1 + 128*j2 (j1 in [0,128) fast, j2 in [0,32));
k = kap2 + 32*kap1. Batch b = 128*t + 32*g + s (t chunk of 128, g K-pack
group, s in [0,32)). Host does all layout marshalling (numpy index shuffles);
device sees only contiguous [128, X] DMAs.
"""
import numpy as np
import ml_dtypes

import concourse.mybir as mybir
import concourse.tile as tile
from concourse import bacc
from concourse.bass_utils import run_bass_kernel_spmd

NG = 4096                 # global matrix dimension; N = NG*NG
N = NG * NG
NCORES = 8
BPC = NG // NCORES        # 512 signals per core per launch
NCHUNK = 4                # chunks of 128 signals
import os as _os
N_WARMUP = int(_os.environ.get("N_WARMUP", "8"))

_F32 = mybir.dt.float32
_BF16 = mybir.dt.bfloat16
_NPBF16 = ml_dtypes.bfloat16

# ---------------------------------------------------------------------------
# constants (host-side numpy)
# ---------------------------------------------------------------------------

_consts_cache = None


def _make_consts():
    """bdc: [128, 512] bf16  (p = 32g+j2; cols = 256*srcpl + 128*pl' + 32g+kap2)
    bm:  [128, 32, 3, 128] bf16 (p = j1; kap2, {Br,Bi,-Bi}, kap1)"""
    global _consts_cache
    if _consts_cache is not None:
        return _consts_cache
    j2 = np.arange(32)
    W32 = np.exp(-2j * np.pi * np.outer(j2, j2) / 32)
    I4 = np.eye(4)
    BDr = np.kron(I4, W32.real)
    BDi = np.kron(I4, W32.imag)
    bdc = np.concatenate([
        np.concatenate([BDr, BDi], axis=1),     # applied to Sr
        np.concatenate([-BDi, BDr], axis=1),    # applied to Si
    ], axis=1).astype(_NPBF16)                  # [128, 512]

    j1 = np.arange(128)
    W128 = np.exp(-2j * np.pi * np.outer(j1, j1) / 128)
    bm = np.zeros((128, 32, 3, 128), np.float32)
    for kap2 in range(32):
        B = np.exp(-2j * np.pi * j1 * kap2 / 4096)[:, None] * W128  # [j1][kap1]
        bm[:, kap2, 0] = B.real
        bm[:, kap2, 1] = B.imag
        bm[:, kap2, 2] = -B.imag
    bm = bm.astype(_NPBF16)
    _consts_cache = (np.ascontiguousarray(bdc), np.ascontiguousarray(bm))
    return _consts_cache


_tw_cache = None


def _global_twiddle():
    """exp(-2pi i k2g*j1g / N) as complex64 [NG, NG] (k2g rows)."""
    global _tw_cache
    if _tw_cache is None:
        k = np.arange(NG, dtype=np.float64)
        phase = np.outer(k, k) * (-2.0 * np.pi / N)
        _tw_cache = np.exp(1j * phase).astype(np.complex64)
    return _tw_cache


# ---------------------------------------------------------------------------
# marshalling (host)
# ---------------------------------------------------------------------------

def _marshal_in(Vre, Vim):
    """Vre/Vim: [4096 f][512 b] f32 planes -> in2 [4,4,128,2,8,128] bf16
    (t, q, p=32g+j2, pl, s8, j1); s = 8q + s8."""
    out = np.empty((NCHUNK, 4, 128, 2, 8, 128), _NPBF16)
    for pl, V in ((0, Vre), (1, Vim)):
        A = V.reshape(32, 128, 4, 4, 4, 8)     # j2, j1, t, g, q, s8
        out[:, :, :, pl] = (
            A.transpose(2, 4, 3, 0, 5, 1)       # t, q, g, j2, s8, j1
            .reshape(NCHUNK, 4, 128, 8, 128)
            .astype(_NPBF16)
        )
    return out


def _unmarshal_out(O):
    """out2 [2,16,128,2,512] bf16 (sc, kp, kap1, pl, n2; n2 = 256u+128c2+4s+g)
    -> (Fre, Fim) planes [4096 k][512 b] f32."""
    O8 = O.reshape(2, 16, 128, 2, 2, 2, 32, 4)  # sc, kp, kap1, pl, u, c2, s, g
    # kap2 = 2*kp+u ; k = 32*kap1 + kap2 ; b = 256*sc + 128*c2 + 32*g + s
    P = np.ascontiguousarray(
        O8.transpose(3, 2, 1, 4, 0, 5, 7, 6)    # pl, kap1, kp, u, sc, c2, g, s
    ).reshape(2, NG, BPC).astype(np.float32)
    return P[0], P[1]


# ---------------------------------------------------------------------------
# device kernel (Bass/Tile), shared by both launches
# ---------------------------------------------------------------------------

_nc_cache = None


def _build_nc():
    global _nc_cache
    if _nc_cache is not None:
        return _nc_cache

    nc = bacc.Bacc(trn_type="TRN2")
    # in layout: [t, q, p = 32g+j2, (pl, s8, j1)]
    in_d = nc.dram_tensor("in2", [NCHUNK, 4, 128, 2, 8, 128], _BF16,
                          kind="ExternalInput")
    bdc_d = nc.dram_tensor("bdc", [128, 512], _BF16, kind="ExternalInput")
    bm_d = nc.dram_tensor("bm", [128, 32, 3, 128], _BF16, kind="ExternalInput")
    # out layout: [superchunk, kap2pair, kap1, pl, n2], n2 = 256u+128c2+4s+g
    out_d = nc.dram_tensor("out2", [NCHUNK // 2, 16, 128, 2, 512], _BF16,
                           kind="ExternalOutput")

    with tile.TileContext(nc) as tc:
        with (
            tc.tile_pool(name="consts", bufs=1) as cpool,
            tc.tile_pool(name="inp", bufs=6) as inpool,
            tc.tile_pool(name="inh", bufs=8) as inhpool,
            tc.tile_pool(name="tp", bufs=2) as tpool,
            tc.tile_pool(name="outp", bufs=16) as outpool,
            tc.tile_pool(name="scr", bufs=4) as scrpool,
            tc.tile_pool(name="pA", bufs=4, space="PSUM") as pA,
            tc.tile_pool(name="pB", bufs=4, space="PSUM") as pB,
        ):
            # --- greedy balancing of PSUM evacuations over DVE + ACT -------
            # (GPSIMD/Pool cannot access PSUM on TRN2 — BIR verifier)
            eng_load = {"v": 0.0, "s": 0.0}
            # rough ns per [128, cols] op: per-elem rate + fixed overhead
            eng_cost = {
                "v": lambda cols: cols * 1.042 + 190.0,   # DVE @0.96GHz
                "s": lambda cols: cols * 0.833 + 230.0,   # ACT @1.2GHz
            }

            def _pick(cols):
                e = min(eng_load, key=lambda k: eng_load[k] + eng_cost[k](cols))
                eng_load[e] += eng_cost[e](cols)
                return e

            def evac(out_ap, in_ap, cols, fast=False, allow=None):
                if allow:
                    e = min(allow, key=lambda k: eng_load[k] + eng_cost[k](cols))
                    eng_load[e] += eng_cost[e](cols)
                else:
                    e = _pick(cols)
                if e == "v":
                    nc.vector.tensor_copy(out_ap, in_ap)
                else:
                    nc.scalar.copy(out_ap, in_ap)

            # --- resident constants ---------------------------------------
            bdc_t = cpool.tile([128, 512], _BF16, tag="bdc")
            bm_t = cpool.tile([128, 32, 3, 128], _BF16, tag="bm")

            in_tiles = {}

            def load_input(t, q, split=1):
                """split=2 halves the DMA (earlier first matmul at startup)."""
                if split == 1:
                    it = inpool.tile([128, 2, 8, 128], _BF16, tag="in")
                    nc.sync.dma_start(it[:], in_d[t, q])
                    in_tiles[(t, q)] = [it]
                else:
                    tiles = []
                    for hh in range(2):
                        it = inhpool.tile([128, 2, 4, 128], _BF16, tag="inh")
                        nc.sync.dma_start(it[:], in_d[t, q, :, :, 4 * hh:4 * hh + 4])
                        tiles.append(it)
                    in_tiles[(t, q)] = tiles

            def slice_in(t, s):
                """stationary [128, 128] slices (re, im) for s within chunk t."""
                tiles = in_tiles[(t, s // 8)]
                r = s % 8
                it = tiles[0] if len(tiles) == 1 or r < 4 else tiles[1]
                r = r if len(tiles) == 1 else r % 4
                return it[:, 0, r], it[:, 1, r]

            # PE warmup: dependency-free matmuls on a memset tile keep the
            # tensor engine continuously busy from t~0.3us, so the p-state
            # ramp (half clock for the first 3us of busy) completes before
            # the first real matmul instead of slowing it down.
            wz = scrpool.tile([128, 512], _BF16, tag="warm")
            nc.vector.memset(wz[:], 0.0)
            wbank = pA.tile([128, 512], _F32, tag="psA")
            for _ in range(N_WARMUP):
                nc.tensor.matmul(wbank[:], wz[:, 0:128], wz[:],
                                 start=True, stop=True)

            # DMA order: bdc first (needed by the first matmul), then input
            # chunks 0-2; the three bm pieces ride between the chunk-2
            # quarters (the A0,A1,A2,B0,A3,B1 stage order gives them slack).
            nc.sync.dma_start(bdc_t[:], bdc_d.ap())
            for q in range(4):
                load_input(0, q, split=2)
            for q in range(4):
                load_input(1, q)
            for q in range(4):
                load_input(2, q)
                if q == 1:
                    nc.sync.dma_start(bm_t[:, 0:8], bm_d.ap()[:, 0:8])
            nc.sync.dma_start(bm_t[:, 8:16], bm_d.ap()[:, 8:16])
            nc.sync.dma_start(bm_t[:, 16:32], bm_d.ap()[:, 16:32])
            for q in range(4):
                load_input(3, q)

            tts = []
            for sc in range(NCHUNK // 2):
                tt = tpool.tile([128, 2, 32, 2, 4, 32], _BF16, tag="tt")
                tts.append((tt, tt.rearrange("p a b c d e -> p (a b c d e)")))

            def stage_a(t):
                # ---- fused stage A + transpose: per s:
                #   psum[j1, pl*128 + 32g+kap2] += S_sl.T @ bdc[pl-combo]
                tt, ttf = tts[t // 2]
                c2 = t % 2
                for sp in range(16):             # s-pairs within chunk
                    pool_a = pA if sp % 2 == 0 else pB
                    bank = pool_a.tile([128, 512], _F32,
                                       tag="psA" if sp % 2 == 0 else "psB")
                    for e in range(2):
                        sl = 2 * sp + e          # s within chunk
                        sre, sim = slice_in(t, sl)
                        ys = slice(256 * e, 256 * e + 256)
                        nc.tensor.matmul(bank[:, ys], sre,
                                         bdc_t[:, 0:256], start=True, stop=False)
                        nc.tensor.matmul(bank[:, ys], sim,
                                         bdc_t[:, 256:512], start=False, stop=True)
                    off = (c2 * 32 + 2 * sp) * 256
                    if t == 3 and sp >= 8:
                        # the last banks gate stage B (it reads all of tt):
                        # strict bank-level alternation keeps both queues
                        # short so the final evac lands promptly
                        if sp % 2 == 0:
                            nc.vector.tensor_copy(ttf[:, off:off + 512], bank[:])
                            eng_load["v"] += eng_cost["v"](512)
                        else:
                            nc.scalar.copy(ttf[:, off:off + 512], bank[:])
                            eng_load["s"] += eng_cost["s"](512)
                    else:
                        evac(ttf[:, off:off + 512], bank[:], 512)

            def stage_b(sc):
                # ---- stage B: radix-128, per-kap2 twiddled weights, N=256
                tt, ttf = tts[sc]
                for kp in range(16):             # kap2 pairs
                    tail = (sc == NCHUNK // 2 - 1) and kp == 15
                    kp_allow = None
                    if tail:
                        # progressive split of the final kp: each sub-group's
                        # evac+DMA drains while the next sub-group's matmuls
                        # run, so only a tiny store chain trails the last
                        # matmul
                        ot = outpool.tile([128, 2, 512], _BF16, tag="out")
                        for u in range(2):
                            kap2 = 2 * kp + u
                            us = slice(256 * u, 256 * u + 256)
                            yru = pA.tile([128, 256], _F32, tag="psA")
                            yiu = pA.tile([128, 256], _F32, tag="psA")
                            trs = tt[:, :, :, 0, :, kap2]
                            tis = tt[:, :, :, 1, :, kap2]
                            br = bm_t[:, kap2, 0]
                            bi = bm_t[:, kap2, 1]
                            bni = bm_t[:, kap2, 2]
                            nc.tensor.matmul(yru[:], br, trs, start=True, stop=False)
                            nc.tensor.matmul(yiu[:], br, tis, start=True, stop=False)
                            nc.tensor.matmul(yru[:], bni, tis, start=False, stop=True)
                            nc.tensor.matmul(yiu[:], bi, trs, start=False, stop=True)
                            nc.vector.tensor_copy(ot[:, 0, us], yru[:])
                            nc.scalar.copy(ot[:, 1, us], yiu[:])
                        nc.sync.dma_start(out_d[sc, kp], ot[:])
                        continue
                    # the first kps after the A3|B1 boundary borrow the idle
                    # pA banks; elsewhere alternate pools (pA idles during
                    # stage B) to double the kp pipeline depth
                    boundary = sc == 1 and kp < 2
                    pool = pA if (boundary or kp % 2 == 1) else pB
                    yr = pool.tile([128, 512], _F32, tag="psA" if pool is pA else "psB")
                    yi = pool.tile([128, 512], _F32, tag="psA" if pool is pA else "psB")
                    if boundary:
                        # c2-split matmul order: the c2=0 half of tt(sc1) was
                        # written by A2 (long done), so these matmuls bridge
                        # the bubble while A3's last evacuations land
                        for c2v in range(2):
                            for u in range(2):
                                kap2 = 2 * kp + u
                                ys = slice(256 * u + 128 * c2v,
                                           256 * u + 128 * c2v + 128)
                                trs = tt[:, c2v, :, 0, :, kap2]
                                tis = tt[:, c2v, :, 1, :, kap2]
                                br = bm_t[:, kap2, 0]
                                bi = bm_t[:, kap2, 1]
                                bni = bm_t[:, kap2, 2]
                                nc.tensor.matmul(yr[:, ys], br, trs,
                                                 start=True, stop=False)
                                nc.tensor.matmul(yi[:, ys], br, tis,
                                                 start=True, stop=False)
                                nc.tensor.matmul(yr[:, ys], bni, tis,
                                                 start=False, stop=True)
                                nc.tensor.matmul(yi[:, ys], bi, trs,
                                                 start=False, stop=True)
                    else:
                        for u in range(2):
                            kap2 = 2 * kp + u
                            ys = slice(256 * u, 256 * u + 256)
                            trs = tt[:, :, :, 0, :, kap2]
                            tis = tt[:, :, :, 1, :, kap2]
                            br = bm_t[:, kap2, 0]
                            bi = bm_t[:, kap2, 1]
                            bni = bm_t[:, kap2, 2]
                            nc.tensor.matmul(yr[:, ys], br, trs, start=True, stop=False)
                            nc.tensor.matmul(yi[:, ys], br, tis, start=True, stop=False)
                            nc.tensor.matmul(yr[:, ys], bni, tis, start=False, stop=True)
                            nc.tensor.matmul(yi[:, ys], bi, trs, start=False, stop=True)
                    # evac + store
                    ot = outpool.tile([128, 2, 512], _BF16, tag="out")
                    evac(ot[:, 0], yr[:], 512, allow=kp_allow)
                    evac(ot[:, 1], yi[:], 512, allow=kp_allow)
                    nc.sync.dma_start(out_d[sc, kp], ot[:])

            # A2 runs before B0: every weight/input DMA deadline gains the
            # width of one stage-A chunk, removing the early supply crunch
            stage_a(0)
            stage_a(1)
            stage_a(2)
            stage_b(0)
            stage_a(3)
            stage_b(1)

    nc.finalize()
    _nc_cache = nc
    return nc


# ---------------------------------------------------------------------------
# launch helper
# ---------------------------------------------------------------------------

_last_exec_ns = None


def last_exec_time_ns():
    """Sum of HW exec times (ns) of the launches in the last kernel() call,
    when KERNEL_TRACE=1 was set and NTFF profiling is available. None otherwise."""
    return _last_exec_ns


def predicted_exec_time_ns():
    """Cost-model (TimelineSim) predicted HW exec time for both launches, ns."""
    from concourse.timeline_sim import TimelineSim
    nc = _build_nc()
    return int(2 * TimelineSim(nc).simulate())


def _run_launch(cols_re, cols_im):
    """cols_re/cols_im: list of 8 planes [4096 f][512 b] f32.
    Returns list of 8 (Fre, Fim) planes [4096 k][512 b]."""
    global _last_exec_ns
    import os
    nc = _build_nc()
    bdc, bm = _make_consts()
    in_maps = []
    for c in range(NCORES):
        in_maps.append({
            "in2": _marshal_in(cols_re[c], cols_im[c]),
            "bdc": bdc, "bm": bm,
        })
    trace = bool(os.environ.get("KERNEL_TRACE"))
    try:
        res = run_bass_kernel_spmd(nc, in_maps, core_ids=list(range(NCORES)),
                                   trace=trace)
    except ModuleNotFoundError:
        # NTFF profiling hook unavailable under this axon client; run untraced.
        res = run_bass_kernel_spmd(nc, in_maps, core_ids=list(range(NCORES)))
    if trace and getattr(res, "exec_time_ns", None) is not None:
        _last_exec_ns = (_last_exec_ns or 0) + res.exec_time_ns
    return [_unmarshal_out(np.asarray(res.results[c]["out2"]))
            for c in range(NCORES)]


# ---------------------------------------------------------------------------
# public entry point
# ---------------------------------------------------------------------------

def kernel(x: np.ndarray) -> np.ndarray:
    """x: [N, 2] float32 (re, im). Returns FFT(x) as [N, 2] float32."""
    global _last_exec_ns
    _last_exec_ns = None
    x = np.asarray(x)
    Are = np.ascontiguousarray(x[:, 0].reshape(NG, NG))  # [j2g][j1g]
    Aim = np.ascontiguousarray(x[:, 1].reshape(NG, NG))

    # launch 1: FFT over rows (j2g) for each column j1g
    cols_re = [np.ascontiguousarray(Are[:, BPC * c:BPC * (c + 1)]) for c in range(NCORES)]
    cols_im = [np.ascontiguousarray(Aim[:, BPC * c:BPC * (c + 1)]) for c in range(NCORES)]
    l1 = _run_launch(cols_re, cols_im)

    # host: assemble F [k2g][j1g], twiddle, transpose-exchange
    F = np.empty((NG, NG), np.complex64)
    for c in range(NCORES):
        fre, fim = l1[c]
        F[:, BPC * c:BPC * (c + 1)] = fre + 1j * fim
    F *= _global_twiddle()

    # launch 2: FFT over j1g for each row k2g; core d gets rows [512d, 512(d+1))
    cols_re2 = []
    cols_im2 = []
    for d in range(NCORES):
        block = F[BPC * d:BPC * (d + 1), :].T      # [j1g][k2g-local]
        cols_re2.append(np.ascontiguousarray(block.real))
        cols_im2.append(np.ascontiguousarray(block.imag))
    l2 = _run_launch(cols_re2, cols_im2)

    # assemble Xmat [k1g][k2g]; out flat index k = 4096*k1g + k2g
    out = np.empty((NG, NG, 2), np.float32)
    for d in range(NCORES):
        rre, rim = l2[d]
        out[:, BPC * d:BPC * (d + 1), 0] = rre
        out[:, BPC * d:BPC * (d + 1), 1] = rim
    return out.reshape(N, 2)


# revision 106
# speedup vs baseline: 1.0581x; 1.0427x over previous
"""Distributed FFT (N = 2^24 complex points) on 8 Trainium2 NeuronCores.

Four-step (Cooley-Tukey) decomposition N = 4096 x 4096:
  launch 1: per global column j1g, FFT_4096 over j2g      (batch parallel over j1g)
  host:     global twiddle wN^{j1g*k2g} + transpose exchange
  launch 2: per global row k2g, FFT_4096 over j1g         (batch parallel over k2g)

Both launches run the SAME compiled SPMD kernel on all 8 cores: a batch of
512 local FFT_4096 per core. Each FFT_4096 = radix-32 stage (block-diag 4x
packed over the contraction axis, K=128) fused with its inter-stage transpose
(data-stationary matmul: psum[j1, :] += S_slice.T @ [[Wr|Wi],[-Wi|Wr]]),
then a radix-128 stage whose twiddle exp(-2pi i j1 kap2/4096) is folded into
32 per-kap2 weight matrices {Br, Bi, -Bi}.

All wire traffic (inputs, weights, outputs) is bfloat16: the kernel is
DMA-bound and bf16 halves HBM bytes while the PE runs bf16 matmuls at the
same 1 column/cycle as f32r. Host-side marshalling lays every DRAM tensor
out so each DMA moves >=2KB contiguous runs per partition (large
descriptors). PSUM (fp32) evacuations are load-balanced across DVE and
Activation (GPSIMD cannot access PSUM). Stage order A0,A1,A2,B0,A3,B1
relaxes every DMA deadline; warmup matmuls pre-ramp the PE p-state; the
first kps after the A3|B1 boundary split their matmuls by c2-half so the
PE bridges the final evacuation latency. Both stages alternate psum
allocations across the two pools (the other stage's pool idles), doubling
the in-flight bank depth and absorbing evacuation-latency jitter.

Local FFT_4096 digits: f = j1 + 128*j2 (j1 in [0,128) fast, j2 in [0,32));
k = kap2 + 32*kap1. Batch b = 128*t + 32*g + s (t chunk of 128, g K-pack
group, s in [0,32)). Host does all layout marshalling (numpy index shuffles);
device sees only contiguous [128, X] DMAs.
"""
import numpy as np
import ml_dtypes

import concourse.mybir as mybir
import concourse.tile as tile
from concourse import bacc
from concourse.bass_utils import run_bass_kernel_spmd

NG = 4096                 # global matrix dimension; N = NG*NG
N = NG * NG
NCORES = 8
BPC = NG // NCORES        # 512 signals per core per launch
NCHUNK = 4                # chunks of 128 signals
import os as _os
N_WARMUP = int(_os.environ.get("N_WARMUP", "8"))

_F32 = mybir.dt.float32
_BF16 = mybir.dt.bfloat16
_NPBF16 = ml_dtypes.bfloat16

# ---------------------------------------------------------------------------
# constants (host-side numpy)
# ---------------------------------------------------------------------------

_consts_cache = None


def _make_consts():
    """bdc: [128, 512] bf16  (p = 32g+j2; cols = 256*srcpl + 128*pl' + 32g+kap2)
    bm:  [128, 32, 3, 128] bf16 (p = j1; kap2, {Br,Bi,-Bi}, kap1)"""
    global _consts_cache
    if _consts_cache is not None:
        return _consts_cache
    j2 = np.arange(32)
    W32 = np.exp(-2j * np.pi * np.outer(j2, j2) / 32)
    # dense complex-packed block over (pl, j2) x (pl', kap2); block-diag
    # over g2 (2 signals share the contraction dim)
    blk = np.block([[W32.real, W32.imag], [-W32.imag, W32.real]])  # [64, 64]
    bdc = np.kron(np.eye(2), blk).astype(_NPBF16)                  # [128, 128]

    j1 = np.arange(128)
    W128 = np.exp(-2j * np.pi * np.outer(j1, j1) / 128)
    bm = np.zeros((128, 32, 3, 128), np.float32)
    for kap2 in range(32):
        B = np.exp(-2j * np.pi * j1 * kap2 / 4096)[:, None] * W128  # [j1][kap1]
        bm[:, kap2, 0] = B.real
        bm[:, kap2, 1] = B.imag
        bm[:, kap2, 2] = -B.imag
    bm = bm.astype(_NPBF16)
    _consts_cache = (np.ascontiguousarray(bdc), np.ascontiguousarray(bm))
    return _consts_cache


_tw_cache = None


def _global_twiddle():
    """exp(-2pi i k2g*j1g / N) as complex64 [NG, NG] (k2g rows)."""
    global _tw_cache
    if _tw_cache is None:
        k = np.arange(NG, dtype=np.float64)
        phase = np.outer(k, k) * (-2.0 * np.pi / N)
        _tw_cache = np.exp(1j * phase).astype(np.complex64)
    return _tw_cache


# ---------------------------------------------------------------------------
# marshalling (host)
# ---------------------------------------------------------------------------

def _marshal_in(Vre, Vim):
    """Vre/Vim: [4096 f][512 b] f32 planes -> in2 [4,4,128,16,128] bf16
    (t, q, p=64*g2+32*pl+j2, sl16, j1); signal b = 128t + 2*(16q+sl16) + g2."""
    # [pl, j2, j1, t, sl, g2]
    C = np.stack([V.reshape(32, 128, 4, 64, 2) for V in (Vre, Vim)], axis=0)
    out = (
        C.transpose(3, 4, 5, 0, 1, 2)           # t, sl, g2, pl, j2, j1
        .reshape(NCHUNK, 4, 16, 2, 2, 32, 128)  # t, q, sl16, g2, pl, j2, j1
        .transpose(0, 1, 3, 4, 5, 2, 6)         # t, q, g2, pl, j2, sl16, j1
        .reshape(NCHUNK, 4, 128, 16, 128)
        .astype(_NPBF16)
    )
    return np.ascontiguousarray(out)


def _unmarshal_out(O):
    """out2 [2,16,128,2,512] bf16 (sc, kp, kap1, pl, n2; n2 = 256u + nb,
    nb = 128c2+8bk+2sl+g2) -> (Fre, Fim) planes [4096 k][512 b] f32.
    b = 256*sc + nb ; k = 32*kap1 + 2*kp + u."""
    O8 = O.reshape(2, 16, 128, 2, 2, 256)       # sc, kp, kap1, pl, u, nb
    P = np.ascontiguousarray(
        O8.transpose(3, 2, 1, 4, 0, 5)          # pl, kap1, kp, u, sc, nb
    ).reshape(2, NG, BPC).astype(np.float32)
    return P[0], P[1]


# ---------------------------------------------------------------------------
# device kernel (Bass/Tile), shared by both launches
# ---------------------------------------------------------------------------

_nc_cache = None


def _build_nc():
    global _nc_cache
    if _nc_cache is not None:
        return _nc_cache

    nc = bacc.Bacc(trn_type="TRN2")
    # in layout: [t, q, p = 64g2+32pl+j2, (sl16, j1)]
    in_d = nc.dram_tensor("in2", [NCHUNK, 4, 128, 16, 128], _BF16,
                          kind="ExternalInput")
    bdc_d = nc.dram_tensor("bdc", [128, 128], _BF16, kind="ExternalInput")
    bm_d = nc.dram_tensor("bm", [128, 32, 3, 128], _BF16, kind="ExternalInput")
    # out layout: [superchunk, kap2pair, kap1, pl, n2], n2 = 256u+128c2+4s+g
    out_d = nc.dram_tensor("out2", [NCHUNK // 2, 16, 128, 2, 512], _BF16,
                           kind="ExternalOutput")

    with tile.TileContext(nc) as tc:
        with (
            tc.tile_pool(name="consts", bufs=1) as cpool,
            tc.tile_pool(name="inp", bufs=6) as inpool,
            tc.tile_pool(name="inh", bufs=8) as inhpool,
            tc.tile_pool(name="tp", bufs=2) as tpool,
            tc.tile_pool(name="outp", bufs=16) as outpool,
            tc.tile_pool(name="scr", bufs=4) as scrpool,
            tc.tile_pool(name="pA", bufs=4, space="PSUM") as pA,
            tc.tile_pool(name="pB", bufs=4, space="PSUM") as pB,
        ):
            # --- greedy balancing of PSUM evacuations over DVE + ACT -------
            # (GPSIMD/Pool cannot access PSUM on TRN2 — BIR verifier)
            eng_load = {"v": 0.0, "s": 0.0}
            # rough ns per [128, cols] op: per-elem rate + fixed overhead
            eng_cost = {
                "v": lambda cols: cols * 1.042 + 190.0,   # DVE @0.96GHz
                "s": lambda cols: cols * 0.833 + 230.0,   # ACT @1.2GHz
            }

            def _pick(cols):
                e = min(eng_load, key=lambda k: eng_load[k] + eng_cost[k](cols))
                eng_load[e] += eng_cost[e](cols)
                return e

            def evac(out_ap, in_ap, cols, fast=False, allow=None):
                if allow:
                    e = min(allow, key=lambda k: eng_load[k] + eng_cost[k](cols))
                    eng_load[e] += eng_cost[e](cols)
                else:
                    e = _pick(cols)
                if e == "v":
                    nc.vector.tensor_copy(out_ap, in_ap)
                else:
                    nc.scalar.copy(out_ap, in_ap)

            # --- resident constants ---------------------------------------
            bdc_t = cpool.tile([128, 128], _BF16, tag="bdc")
            bm_t = cpool.tile([128, 32, 3, 128], _BF16, tag="bm")

            in_tiles = {}

            def load_input(t, q, split=1):
                """split=2 halves the DMA (earlier first matmul at startup)."""
                if split == 1:
                    it = inpool.tile([128, 16, 128], _BF16, tag="in")
                    nc.sync.dma_start(it[:], in_d[t, q])
                    in_tiles[(t, q)] = [it]
                else:
                    tiles = []
                    for hh in range(2):
                        it = inhpool.tile([128, 8, 128], _BF16, tag="inh")
                        nc.sync.dma_start(it[:], in_d[t, q, :, 8 * hh:8 * hh + 8])
                        tiles.append(it)
                    in_tiles[(t, q)] = tiles

            def slice_in(t, sl):
                """stationary [128, 128] slice (both planes, 2 signals in K)
                for slot sl within chunk t."""
                tiles = in_tiles[(t, sl // 16)]
                r = sl % 16
                it = tiles[0] if len(tiles) == 1 or r < 8 else tiles[1]
                r = r if len(tiles) == 1 else r % 8
                return it[:, r]

            # PE warmup: dependency-free matmuls on a memset tile keep the
            # tensor engine continuously busy from t~0.3us, so the p-state
            # ramp (half clock for the first 3us of busy) completes before
            # the first real matmul instead of slowing it down.
            wz = scrpool.tile([128, 512], _BF16, tag="warm")
            nc.vector.memset(wz[:], 0.0)
            wbank = pA.tile([128, 512], _F32, tag="psA")
            for _ in range(N_WARMUP):
                nc.tensor.matmul(wbank[:], wz[:, 0:128], wz[:],
                                 start=True, stop=True)

            # DMA order: bdc first (needed by the first matmul), then input
            # chunks 0-2; the three bm pieces ride between the chunk-2
            # quarters (the A0,A1,A2,B0,A3,B1 stage order gives them slack).
            nc.sync.dma_start(bdc_t[:], bdc_d.ap())
            for q in range(4):
                load_input(0, q, split=2)
            for q in range(4):
                load_input(1, q)
            nc.sync.dma_start(bm_t[:, 0:16], bm_d.ap()[:, 0:16])
            for q in range(4):
                load_input(2, q)
            nc.sync.dma_start(bm_t[:, 16:32], bm_d.ap()[:, 16:32])
            for q in range(4):
                load_input(3, q)

            tts = []
            for sc in range(NCHUNK // 2):
                # free dims: (c2, bk16, sl4, g2, pl, kap2)
                tt = tpool.tile([128, 2, 16, 4, 2, 2, 32], _BF16, tag="tt")
                tts.append((tt, tt.rearrange("p a b c d e f -> p (a b c d e f)")))

            def stage_a(t):
                # ---- fused stage A + transpose: per 2-signal slot, ONE
                # dense matmul (planes live in the contraction dim):
                #   psum[j1, (g2, pl', kap2)] = S_slot.T @ bdc
                tt, ttf = tts[t // 2]
                c2 = t % 2
                for bk in range(16):             # banks of 4 slots (8 signals)
                    pool_a = pA if bk % 2 == 0 else pB
                    bank = pool_a.tile([128, 512], _F32,
                                       tag="psA" if bk % 2 == 0 else "psB")
                    for e in range(4):
                        sl = 4 * bk + e          # slot within chunk
                        sdat = slice_in(t, sl)
                        ys = slice(128 * e, 128 * e + 128)
                        nc.tensor.matmul(bank[:, ys], sdat, bdc_t[:],
                                         start=True, stop=True)
                    off = (c2 * 16 + bk) * 512
                    if t == 3 and bk >= 8:
                        # the last banks gate stage B (it reads all of tt):
                        # strict bank-level alternation keeps both queues
                        # short so the final evac lands promptly
                        if bk % 2 == 0:
                            nc.vector.tensor_copy(ttf[:, off:off + 512], bank[:])
                            eng_load["v"] += eng_cost["v"](512)
                        else:
                            nc.scalar.copy(ttf[:, off:off + 512], bank[:])
                            eng_load["s"] += eng_cost["s"](512)
                    else:
                        evac(ttf[:, off:off + 512], bank[:], 512)

            def stage_b(sc, kps):
                # ---- stage B: radix-128, per-kap2 twiddled weights, N=256
                tt, ttf = tts[sc]
                for kp in kps:                   # kap2 pairs
                    tail = (sc == NCHUNK // 2 - 1) and kp == 15
                    kp_allow = None
                    if tail:
                        # progressive split of the final kp: each sub-group's
                        # evac+DMA drains while the next sub-group's matmuls
                        # run, so only a tiny store chain trails the last
                        # matmul
                        ot = outpool.tile([128, 2, 512], _BF16, tag="out")
                        for u in range(2):
                            kap2 = 2 * kp + u
                            us = slice(256 * u, 256 * u + 256)
                            yru = pA.tile([128, 256], _F32, tag="psA")
                            yiu = pA.tile([128, 256], _F32, tag="psA")
                            trs = tt[:, :, :, :, :, 0, kap2]
                            tis = tt[:, :, :, :, :, 1, kap2]
                            br = bm_t[:, kap2, 0]
                            bi = bm_t[:, kap2, 1]
                            bni = bm_t[:, kap2, 2]
                            nc.tensor.matmul(yru[:], br, trs, start=True, stop=False)
                            nc.tensor.matmul(yiu[:], br, tis, start=True, stop=False)
                            nc.tensor.matmul(yru[:], bni, tis, start=False, stop=True)
                            nc.tensor.matmul(yiu[:], bi, trs, start=False, stop=True)
                            nc.vector.tensor_copy(ot[:, 0, us], yru[:])
                            nc.scalar.copy(ot[:, 1, us], yiu[:])
                        nc.sync.dma_start(out_d[sc, kp], ot[:])
                        continue
                    # the first kps after the A3|B1 boundary borrow the idle
                    # pA banks; elsewhere alternate pools (pA idles during
                    # stage B) to double the kp pipeline depth
                    boundary = sc == 1 and kp < 2
                    pool = pA if (boundary or kp % 2 == 1) else pB
                    yr = pool.tile([128, 512], _F32, tag="psA" if pool is pA else "psB")
                    yi = pool.tile([128, 512], _F32, tag="psA" if pool is pA else "psB")
                    if boundary:
                        # c2-split matmul order: the c2=0 half of tt(sc1) was
                        # written by A2 (long done), so these matmuls bridge
                        # the bubble while A3's last evacuations land
                        for c2v in range(2):
                            for u in range(2):
                                kap2 = 2 * kp + u
                                ys = slice(256 * u + 128 * c2v,
                                           256 * u + 128 * c2v + 128)
                                trs = tt[:, c2v, :, :, :, 0, kap2]
                                tis = tt[:, c2v, :, :, :, 1, kap2]
                                br = bm_t[:, kap2, 0]
                                bi = bm_t[:, kap2, 1]
                                bni = bm_t[:, kap2, 2]
                                nc.tensor.matmul(yr[:, ys], br, trs,
                                                 start=True, stop=False)
                                nc.tensor.matmul(yi[:, ys], br, tis,
                                                 start=True, stop=False)
                                nc.tensor.matmul(yr[:, ys], bni, tis,
                                                 start=False, stop=True)
                                nc.tensor.matmul(yi[:, ys], bi, trs,
                                                 start=False, stop=True)
                    else:
                        for u in range(2):
                            kap2 = 2 * kp + u
                            ys = slice(256 * u, 256 * u + 256)
                            trs = tt[:, :, :, :, :, 0, kap2]
                            tis = tt[:, :, :, :, :, 1, kap2]
                            br = bm_t[:, kap2, 0]
                            bi = bm_t[:, kap2, 1]
                            bni = bm_t[:, kap2, 2]
                            nc.tensor.matmul(yr[:, ys], br, trs, start=True, stop=False)
                            nc.tensor.matmul(yi[:, ys], br, tis, start=True, stop=False)
                            nc.tensor.matmul(yr[:, ys], bni, tis, start=False, stop=True)
                            nc.tensor.matmul(yi[:, ys], bi, trs, start=False, stop=True)
                    # evac + store
                    ot = outpool.tile([128, 2, 512], _BF16, tag="out")
                    evac(ot[:, 0], yr[:], 512, allow=kp_allow)
                    evac(ot[:, 1], yi[:], 512, allow=kp_allow)
                    nc.sync.dma_start(out_d[sc, kp], ot[:])

            # interleaved schedule balancing the serial DMA stream against
            # PE dependencies: each stage starts right as its data lands
            stage_a(0)
            stage_a(1)
            stage_b(0, range(0, 8))
            stage_a(2)
            stage_b(0, range(8, 16))
            stage_a(3)
            stage_b(1, range(16))

    nc.finalize()
    _nc_cache = nc
    return nc


# ---------------------------------------------------------------------------
# launch helper
# ---------------------------------------------------------------------------

_last_exec_ns = None


def last_exec_time_ns():
    """Sum of HW exec times (ns) of the launches in the last kernel() call,
    when KERNEL_TRACE=1 was set and NTFF profiling is available. None otherwise."""
    return _last_exec_ns


def predicted_exec_time_ns():
    """Cost-model (TimelineSim) predicted HW exec time for both launches, ns."""
    from concourse.timeline_sim import TimelineSim
    nc = _build_nc()
    return int(2 * TimelineSim(nc).simulate())


def _run_launch(cols_re, cols_im):
    """cols_re/cols_im: list of 8 planes [4096 f][512 b] f32.
    Returns list of 8 (Fre, Fim) planes [4096 k][512 b]."""
    global _last_exec_ns
    import os
    nc = _build_nc()
    bdc, bm = _make_consts()
    in_maps = []
    for c in range(NCORES):
        in_maps.append({
            "in2": _marshal_in(cols_re[c], cols_im[c]),
            "bdc": bdc, "bm": bm,
        })
    trace = bool(os.environ.get("KERNEL_TRACE"))
    try:
        res = run_bass_kernel_spmd(nc, in_maps, core_ids=list(range(NCORES)),
                                   trace=trace)
    except ModuleNotFoundError:
        # NTFF profiling hook unavailable under this axon client; run untraced.
        res = run_bass_kernel_spmd(nc, in_maps, core_ids=list(range(NCORES)))
    if trace and getattr(res, "exec_time_ns", None) is not None:
        _last_exec_ns = (_last_exec_ns or 0) + res.exec_time_ns
    return [_unmarshal_out(np.asarray(res.results[c]["out2"]))
            for c in range(NCORES)]


# ---------------------------------------------------------------------------
# public entry point
# ---------------------------------------------------------------------------

def kernel(x: np.ndarray) -> np.ndarray:
    """x: [N, 2] float32 (re, im). Returns FFT(x) as [N, 2] float32."""
    global _last_exec_ns
    _last_exec_ns = None
    x = np.asarray(x)
    Are = np.ascontiguousarray(x[:, 0].reshape(NG, NG))  # [j2g][j1g]
    Aim = np.ascontiguousarray(x[:, 1].reshape(NG, NG))

    # launch 1: FFT over rows (j2g) for each column j1g
    cols_re = [np.ascontiguousarray(Are[:, BPC * c:BPC * (c + 1)]) for c in range(NCORES)]
    cols_im = [np.ascontiguousarray(Aim[:, BPC * c:BPC * (c + 1)]) for c in range(NCORES)]
    l1 = _run_launch(cols_re, cols_im)

    # host: assemble F [k2g][j1g], twiddle, transpose-exchange
    F = np.empty((NG, NG), np.complex64)
    for c in range(NCORES):
        fre, fim = l1[c]
        F[:, BPC * c:BPC * (c + 1)] = fre + 1j * fim
    F *= _global_twiddle()

    # launch 2: FFT over j1g for each row k2g; core d gets rows [512d, 512(d+1))
    cols_re2 = []
    cols_im2 = []
    for d in range(NCORES):
        block = F[BPC * d:BPC * (d + 1), :].T      # [j1g][k2g-local]
        cols_re2.append(np.ascontiguousarray(block.real))
        cols_im2.append(np.ascontiguousarray(block.imag))
    l2 = _run_launch(cols_re2, cols_im2)

    # assemble Xmat [k1g][k2g]; out flat index k = 4096*k1g + k2g
    out = np.empty((NG, NG, 2), np.float32)
    for d in range(NCORES):
        rre, rim = l2[d]
        out[:, BPC * d:BPC * (d + 1), 0] = rre
        out[:, BPC * d:BPC * (d + 1), 1] = rim
    return out.reshape(N, 2)


# revision 116
# speedup vs baseline: 1.0608x; 1.0026x over previous
"""Distributed FFT (N = 2^24 complex points) on 8 Trainium2 NeuronCores.

Four-step (Cooley-Tukey) decomposition N = 4096 x 4096:
  launch 1: per global column j1g, FFT_4096 over j2g      (batch parallel over j1g)
  host:     global twiddle wN^{j1g*k2g} + transpose exchange
  launch 2: per global row k2g, FFT_4096 over j1g         (batch parallel over k2g)

Both launches run the SAME compiled SPMD kernel on all 8 cores: a batch of
512 local FFT_4096 per core. Each FFT_4096 = radix-32 stage fused with its
inter-stage transpose as ONE dense matmul per 2-signal slot — the
contraction dim packs (2 signals x 2 planes x 32 j2) so the complex 2x2
structure lives inside a dense [[Wr,Wi],[-Wi,Wr]] weight block (single
accumulation pass; bf16 moving has no minimum-width penalty, unlike f32r) —
then a radix-128 stage whose twiddle exp(-2pi i j1 kap2/4096) is folded
into 32 per-kap2 weight matrices {Br, Bi, -Bi}.

All wire traffic (inputs, weights, outputs) is bfloat16: the kernel is
DMA-bound and bf16 halves HBM bytes while the PE runs bf16 matmuls at the
same 1 column/cycle as f32r. Host-side marshalling lays every DRAM tensor
out so each DMA moves >=2KB contiguous runs per partition (large
descriptors). PSUM (fp32) evacuations are load-balanced across DVE and
Activation (GPSIMD cannot access PSUM). Stage order A0,A1,A2,B0,A3,B1
relaxes every DMA deadline; warmup matmuls pre-ramp the PE p-state; the
first kps after the A3|B1 boundary split their matmuls by c2-half so the
PE bridges the final evacuation latency. Both stages alternate psum
allocations across the two pools (the other stage's pool idles), doubling
the in-flight bank depth and absorbing evacuation-latency jitter.

Local FFT_4096 digits: f = j1 + 128*j2 (j1 in [0,128) fast, j2 in [0,32));
k = kap2 + 32*kap1. Batch b = 128*t + 2*slot + g2 (t chunk of 128, slot in
[0,64), g2 the K-pack pair index). Host does all layout marshalling (numpy
index shuffles); device sees only contiguous [128, X] DMAs.
"""
import numpy as np
import ml_dtypes

import concourse.mybir as mybir
import concourse.tile as tile
from concourse import bacc
from concourse.bass_utils import run_bass_kernel_spmd

NG = 4096                 # global matrix dimension; N = NG*NG
N = NG * NG
NCORES = 8
BPC = NG // NCORES        # 512 signals per core per launch
NCHUNK = 4                # chunks of 128 signals
import os as _os
N_WARMUP = int(_os.environ.get("N_WARMUP", "8"))

_F32 = mybir.dt.float32
_BF16 = mybir.dt.bfloat16
_NPBF16 = ml_dtypes.bfloat16

# ---------------------------------------------------------------------------
# constants (host-side numpy)
# ---------------------------------------------------------------------------

_consts_cache = None


def _make_consts():
    """bdc: [128, 512] bf16  (p = 32g+j2; cols = 256*srcpl + 128*pl' + 32g+kap2)
    bm:  [128, 32, 3, 128] bf16 (p = j1; kap2, {Br,Bi,-Bi}, kap1)"""
    global _consts_cache
    if _consts_cache is not None:
        return _consts_cache
    j2 = np.arange(32)
    W32 = np.exp(-2j * np.pi * np.outer(j2, j2) / 32)
    # dense complex-packed block over (pl, j2) x (pl', kap2); block-diag
    # over g2 (2 signals share the contraction dim)
    blk = np.block([[W32.real, W32.imag], [-W32.imag, W32.real]])  # [64, 64]
    bdc = np.kron(np.eye(2), blk).astype(_NPBF16)                  # [128, 128]

    j1 = np.arange(128)
    W128 = np.exp(-2j * np.pi * np.outer(j1, j1) / 128)
    bm = np.zeros((128, 32, 3, 128), np.float32)
    for kap2 in range(32):
        B = np.exp(-2j * np.pi * j1 * kap2 / 4096)[:, None] * W128  # [j1][kap1]
        bm[:, kap2, 0] = B.real
        bm[:, kap2, 1] = B.imag
        bm[:, kap2, 2] = -B.imag
    bm = bm.astype(_NPBF16)
    _consts_cache = (np.ascontiguousarray(bdc), np.ascontiguousarray(bm))
    return _consts_cache


_tw_cache = None


def _global_twiddle():
    """exp(-2pi i k2g*j1g / N) as complex64 [NG, NG] (k2g rows)."""
    global _tw_cache
    if _tw_cache is None:
        k = np.arange(NG, dtype=np.float64)
        phase = np.outer(k, k) * (-2.0 * np.pi / N)
        _tw_cache = np.exp(1j * phase).astype(np.complex64)
    return _tw_cache


# ---------------------------------------------------------------------------
# marshalling (host)
# ---------------------------------------------------------------------------

def _marshal_in(Vre, Vim):
    """Vre/Vim: [4096 f][512 b] f32 planes -> in2 [4,4,128,16,128] bf16
    (t, q, p=64*g2+32*pl+j2, sl16, j1); signal b = 128t + 2*(16q+sl16) + g2."""
    # [pl, j2, j1, t, sl, g2]
    C = np.stack([V.reshape(32, 128, 4, 64, 2) for V in (Vre, Vim)], axis=0)
    out = (
        C.transpose(3, 4, 5, 0, 1, 2)           # t, sl, g2, pl, j2, j1
        .reshape(NCHUNK, 4, 16, 2, 2, 32, 128)  # t, q, sl16, g2, pl, j2, j1
        .transpose(0, 1, 3, 4, 5, 2, 6)         # t, q, g2, pl, j2, sl16, j1
        .reshape(NCHUNK, 4, 128, 16, 128)
        .astype(_NPBF16)
    )
    return np.ascontiguousarray(out)


def _unmarshal_out(O):
    """out2 [2,16,128,2,512] bf16 (sc, kp, kap1, pl, n2; n2 = 256u + nb,
    nb = 128c2+8bk+2sl+g2) -> (Fre, Fim) planes [4096 k][512 b] f32.
    b = 256*sc + nb ; k = 32*kap1 + 2*kp + u."""
    O8 = O.reshape(2, 16, 128, 2, 2, 256)       # sc, kp, kap1, pl, u, nb
    P = np.ascontiguousarray(
        O8.transpose(3, 2, 1, 4, 0, 5)          # pl, kap1, kp, u, sc, nb
    ).reshape(2, NG, BPC).astype(np.float32)
    return P[0], P[1]


# ---------------------------------------------------------------------------
# device kernel (Bass/Tile), shared by both launches
# ---------------------------------------------------------------------------

_nc_cache = None


def _build_nc():
    global _nc_cache
    if _nc_cache is not None:
        return _nc_cache

    nc = bacc.Bacc(trn_type="TRN2")
    # in layout: [t, q, p = 64g2+32pl+j2, (sl16, j1)]
    in_d = nc.dram_tensor("in2", [NCHUNK, 4, 128, 16, 128], _BF16,
                          kind="ExternalInput")
    bdc_d = nc.dram_tensor("bdc", [128, 128], _BF16, kind="ExternalInput")
    bm_d = nc.dram_tensor("bm", [128, 32, 3, 128], _BF16, kind="ExternalInput")
    # out layout: [superchunk, kap2pair, kap1, pl, n2], n2 = 256u+128c2+4s+g
    out_d = nc.dram_tensor("out2", [NCHUNK // 2, 16, 128, 2, 512], _BF16,
                           kind="ExternalOutput")

    with tile.TileContext(nc) as tc:
        with (
            tc.tile_pool(name="consts", bufs=1) as cpool,
            tc.tile_pool(name="inp", bufs=6) as inpool,
            tc.tile_pool(name="inh", bufs=8) as inhpool,
            tc.tile_pool(name="tp", bufs=2) as tpool,
            tc.tile_pool(name="outp", bufs=16) as outpool,
            tc.tile_pool(name="scr", bufs=4) as scrpool,
            tc.tile_pool(name="pA", bufs=4, space="PSUM") as pA,
            tc.tile_pool(name="pB", bufs=4, space="PSUM") as pB,
        ):
            # --- greedy balancing of PSUM evacuations over DVE + ACT -------
            # (GPSIMD/Pool cannot access PSUM on TRN2 — BIR verifier)
            eng_load = {"v": 0.0, "s": 0.0}
            # rough ns per [128, cols] op: per-elem rate + fixed overhead
            eng_cost = {
                "v": lambda cols: cols * 1.042 + 190.0,   # DVE @0.96GHz
                "s": lambda cols: cols * 0.833 + 230.0,   # ACT @1.2GHz
            }

            def _pick(cols):
                e = min(eng_load, key=lambda k: eng_load[k] + eng_cost[k](cols))
                eng_load[e] += eng_cost[e](cols)
                return e

            def evac(out_ap, in_ap, cols, fast=False, allow=None):
                if allow:
                    e = min(allow, key=lambda k: eng_load[k] + eng_cost[k](cols))
                    eng_load[e] += eng_cost[e](cols)
                else:
                    e = _pick(cols)
                if e == "v":
                    nc.vector.tensor_copy(out_ap, in_ap)
                else:
                    nc.scalar.copy(out_ap, in_ap)

            # --- resident constants ---------------------------------------
            bdc_t = cpool.tile([128, 128], _BF16, tag="bdc")
            bm_t = cpool.tile([128, 32, 3, 128], _BF16, tag="bm")

            in_tiles = {}

            def load_input(t, q, split=1):
                """split=2 halves the DMA (earlier first matmul at startup)."""
                if split == 1:
                    it = inpool.tile([128, 16, 128], _BF16, tag="in")
                    nc.sync.dma_start(it[:], in_d[t, q])
                    in_tiles[(t, q)] = [it]
                else:
                    tiles = []
                    for hh in range(2):
                        it = inhpool.tile([128, 8, 128], _BF16, tag="inh")
                        nc.sync.dma_start(it[:], in_d[t, q, :, 8 * hh:8 * hh + 8])
                        tiles.append(it)
                    in_tiles[(t, q)] = tiles

            def slice_in(t, sl):
                """stationary [128, 128] slice (both planes, 2 signals in K)
                for slot sl within chunk t."""
                tiles = in_tiles[(t, sl // 16)]
                r = sl % 16
                it = tiles[0] if len(tiles) == 1 or r < 8 else tiles[1]
                r = r if len(tiles) == 1 else r % 8
                return it[:, r]

            # PE warmup: dependency-free matmuls on a memset tile keep the
            # tensor engine continuously busy from t~0.3us, so the p-state
            # ramp (half clock for the first 3us of busy) completes before
            # the first real matmul instead of slowing it down.
            wz = scrpool.tile([128, 512], _BF16, tag="warm")
            nc.vector.memset(wz[:], 0.0)
            wbank = pA.tile([128, 512], _F32, tag="psA")
            for _ in range(N_WARMUP):
                nc.tensor.matmul(wbank[:], wz[:, 0:128], wz[:],
                                 start=True, stop=True)

            # DMA order: bdc first (needed by the first matmul), then input
            # chunks 0-2; the three bm pieces ride between the chunk-2
            # quarters (the A0,A1,A2,B0,A3,B1 stage order gives them slack).
            load_input(0, 0, split=2)
            nc.sync.dma_start(bdc_t[:], bdc_d.ap())
            for q in range(1, 4):
                load_input(0, q, split=2)
            for q in range(4):
                load_input(1, q)
            for q in range(4):
                load_input(2, q)
                if q == 1:
                    nc.sync.dma_start(bm_t[:, 0:8], bm_d.ap()[:, 0:8])
            nc.sync.dma_start(bm_t[:, 8:16], bm_d.ap()[:, 8:16])
            nc.sync.dma_start(bm_t[:, 16:32], bm_d.ap()[:, 16:32])
            for q in range(4):
                load_input(3, q)

            tts = []
            for sc in range(NCHUNK // 2):
                # free dims: (c2, bk16, sl4, g2, pl, kap2)
                tt = tpool.tile([128, 2, 16, 4, 2, 2, 32], _BF16, tag="tt")
                tts.append((tt, tt.rearrange("p a b c d e f -> p (a b c d e f)")))

            def stage_a(t):
                # ---- fused stage A + transpose: per 2-signal slot, ONE
                # dense matmul (planes live in the contraction dim):
                #   psum[j1, (g2, pl', kap2)] = S_slot.T @ bdc
                tt, ttf = tts[t // 2]
                c2 = t % 2
                for bk in range(16):             # banks of 4 slots (8 signals)
                    pool_a = pA if bk % 2 == 0 else pB
                    bank = pool_a.tile([128, 512], _F32,
                                       tag="psA" if bk % 2 == 0 else "psB")
                    for e in range(4):
                        sl = 4 * bk + e          # slot within chunk
                        sdat = slice_in(t, sl)
                        ys = slice(128 * e, 128 * e + 128)
                        nc.tensor.matmul(bank[:, ys], sdat, bdc_t[:],
                                         start=True, stop=True)
                    off = (c2 * 16 + bk) * 512
                    if t == 3 and bk >= 8:
                        # the last banks gate stage B (it reads all of tt):
                        # strict bank-level alternation keeps both queues
                        # short so the final evac lands promptly
                        if bk % 2 == 0:
                            nc.vector.tensor_copy(ttf[:, off:off + 512], bank[:])
                            eng_load["v"] += eng_cost["v"](512)
                        else:
                            nc.scalar.copy(ttf[:, off:off + 512], bank[:])
                            eng_load["s"] += eng_cost["s"](512)
                    else:
                        evac(ttf[:, off:off + 512], bank[:], 512)

            def stage_b(sc):
                # ---- stage B: radix-128, per-kap2 twiddled weights, N=256
                tt, ttf = tts[sc]
                for kp in range(16):             # kap2 pairs
                    tail = (sc == NCHUNK // 2 - 1) and kp == 15
                    kp_allow = None
                    if tail:
                        # progressive split of the final kp: each sub-group's
                        # evac+DMA drains while the next sub-group's matmuls
                        # run, so only a tiny store chain trails the last
                        # matmul
                        ot = outpool.tile([128, 2, 512], _BF16, tag="out")
                        for u in range(2):
                            kap2 = 2 * kp + u
                            us = slice(256 * u, 256 * u + 256)
                            yru = pA.tile([128, 256], _F32, tag="psA")
                            yiu = pA.tile([128, 256], _F32, tag="psA")
                            trs = tt[:, :, :, :, :, 0, kap2]
                            tis = tt[:, :, :, :, :, 1, kap2]
                            br = bm_t[:, kap2, 0]
                            bi = bm_t[:, kap2, 1]
                            bni = bm_t[:, kap2, 2]
                            nc.tensor.matmul(yru[:], br, trs, start=True, stop=False)
                            nc.tensor.matmul(yiu[:], br, tis, start=True, stop=False)
                            nc.tensor.matmul(yru[:], bni, tis, start=False, stop=True)
                            nc.tensor.matmul(yiu[:], bi, trs, start=False, stop=True)
                            nc.vector.tensor_copy(ot[:, 0, us], yru[:])
                            nc.scalar.copy(ot[:, 1, us], yiu[:])
                        nc.sync.dma_start(out_d[sc, kp], ot[:])
                        continue
                    # the first kps after the A3|B1 boundary borrow the idle
                    # pA banks; elsewhere alternate pools (pA idles during
                    # stage B) to double the kp pipeline depth
                    boundary = sc == 1 and kp < 2
                    pool = pA if (boundary or kp % 2 == 1) else pB
                    yr = pool.tile([128, 512], _F32, tag="psA" if pool is pA else "psB")
                    yi = pool.tile([128, 512], _F32, tag="psA" if pool is pA else "psB")
                    if boundary:
                        # c2-split matmul order: the c2=0 half of tt(sc1) was
                        # written by A2 (long done), so these matmuls bridge
                        # the bubble while A3's last evacuations land
                        for c2v in range(2):
                            for u in range(2):
                                kap2 = 2 * kp + u
                                ys = slice(256 * u + 128 * c2v,
                                           256 * u + 128 * c2v + 128)
                                trs = tt[:, c2v, :, :, :, 0, kap2]
                                tis = tt[:, c2v, :, :, :, 1, kap2]
                                br = bm_t[:, kap2, 0]
                                bi = bm_t[:, kap2, 1]
                                bni = bm_t[:, kap2, 2]
                                nc.tensor.matmul(yr[:, ys], br, trs,
                                                 start=True, stop=False)
                                nc.tensor.matmul(yi[:, ys], br, tis,
                                                 start=True, stop=False)
                                nc.tensor.matmul(yr[:, ys], bni, tis,
                                                 start=False, stop=True)
                                nc.tensor.matmul(yi[:, ys], bi, trs,
                                                 start=False, stop=True)
                    else:
                        for u in range(2):
                            kap2 = 2 * kp + u
                            ys = slice(256 * u, 256 * u + 256)
                            trs = tt[:, :, :, :, :, 0, kap2]
                            tis = tt[:, :, :, :, :, 1, kap2]
                            br = bm_t[:, kap2, 0]
                            bi = bm_t[:, kap2, 1]
                            bni = bm_t[:, kap2, 2]
                            nc.tensor.matmul(yr[:, ys], br, trs, start=True, stop=False)
                            nc.tensor.matmul(yi[:, ys], br, tis, start=True, stop=False)
                            nc.tensor.matmul(yr[:, ys], bni, tis, start=False, stop=True)
                            nc.tensor.matmul(yi[:, ys], bi, trs, start=False, stop=True)
                    # evac + store
                    ot = outpool.tile([128, 2, 512], _BF16, tag="out")
                    evac(ot[:, 0], yr[:], 512, allow=kp_allow)
                    evac(ot[:, 1], yi[:], 512, allow=kp_allow)
                    nc.sync.dma_start(out_d[sc, kp], ot[:])

            # A2 runs before B0: every weight/input DMA deadline gains the
            # width of one stage-A chunk, removing the early supply crunch
            stage_a(0)
            stage_a(1)
            stage_a(2)
            stage_b(0)
            stage_a(3)
            stage_b(1)

    nc.finalize()
    _nc_cache = nc
    return nc


# ---------------------------------------------------------------------------
# launch helper
# ---------------------------------------------------------------------------

_last_exec_ns = None


def last_exec_time_ns():
    """Sum of HW exec times (ns) of the launches in the last kernel() call,
    when KERNEL_TRACE=1 was set and NTFF profiling is available. None otherwise."""
    return _last_exec_ns


def predicted_exec_time_ns():
    """Cost-model (TimelineSim) predicted HW exec time for both launches, ns."""
    from concourse.timeline_sim import TimelineSim
    nc = _build_nc()
    return int(2 * TimelineSim(nc).simulate())


def _run_launch(cols_re, cols_im):
    """cols_re/cols_im: list of 8 planes [4096 f][512 b] f32.
    Returns list of 8 (Fre, Fim) planes [4096 k][512 b]."""
    global _last_exec_ns
    import os
    nc = _build_nc()
    bdc, bm = _make_consts()
    in_maps = []
    for c in range(NCORES):
        in_maps.append({
            "in2": _marshal_in(cols_re[c], cols_im[c]),
            "bdc": bdc, "bm": bm,
        })
    trace = bool(os.environ.get("KERNEL_TRACE"))
    try:
        res = run_bass_kernel_spmd(nc, in_maps, core_ids=list(range(NCORES)),
                                   trace=trace)
    except ModuleNotFoundError:
        # NTFF profiling hook unavailable under this axon client; run untraced.
        res = run_bass_kernel_spmd(nc, in_maps, core_ids=list(range(NCORES)))
    if trace and getattr(res, "exec_time_ns", None) is not None:
        _last_exec_ns = (_last_exec_ns or 0) + res.exec_time_ns
    return [_unmarshal_out(np.asarray(res.results[c]["out2"]))
            for c in range(NCORES)]


# ---------------------------------------------------------------------------
# public entry point
# ---------------------------------------------------------------------------

def kernel(x: np.ndarray) -> np.ndarray:
    """x: [N, 2] float32 (re, im). Returns FFT(x) as [N, 2] float32."""
    global _last_exec_ns
    _last_exec_ns = None
    x = np.asarray(x)
    Are = np.ascontiguousarray(x[:, 0].reshape(NG, NG))  # [j2g][j1g]
    Aim = np.ascontiguousarray(x[:, 1].reshape(NG, NG))

    # launch 1: FFT over rows (j2g) for each column j1g
    cols_re = [np.ascontiguousarray(Are[:, BPC * c:BPC * (c + 1)]) for c in range(NCORES)]
    cols_im = [np.ascontiguousarray(Aim[:, BPC * c:BPC * (c + 1)]) for c in range(NCORES)]
    l1 = _run_launch(cols_re, cols_im)

    # host: assemble F [k2g][j1g], twiddle, transpose-exchange
    F = np.empty((NG, NG), np.complex64)
    for c in range(NCORES):
        fre, fim = l1[c]
        F[:, BPC * c:BPC * (c + 1)] = fre + 1j * fim
    F *= _global_twiddle()

    # launch 2: FFT over j1g for each row k2g; core d gets rows [512d, 512(d+1))
    cols_re2 = []
    cols_im2 = []
    for d in range(NCORES):
        block = F[BPC * d:BPC * (d + 1), :].T      # [j1g][k2g-local]
        cols_re2.append(np.ascontiguousarray(block.real))
        cols_im2.append(np.ascontiguousarray(block.imag))
    l2 = _run_launch(cols_re2, cols_im2)

    # assemble Xmat [k1g][k2g]; out flat index k = 4096*k1g + k2g
    out = np.empty((NG, NG, 2), np.float32)
    for d in range(NCORES):
        rre, rim = l2[d]
        out[:, BPC * d:BPC * (d + 1), 0] = rre
        out[:, BPC * d:BPC * (d + 1), 1] = rim
    return out.reshape(N, 2)


# revision 124
# speedup vs baseline: 1.0635x; 1.0026x over previous
"""Distributed FFT (N = 2^24 complex points) on 8 Trainium2 NeuronCores.

Four-step (Cooley-Tukey) decomposition N = 4096 x 4096:
  launch 1: per global column j1g, FFT_4096 over j2g      (batch parallel over j1g)
  host:     global twiddle wN^{j1g*k2g} + transpose exchange
  launch 2: per global row k2g, FFT_4096 over j1g         (batch parallel over k2g)

Both launches run the SAME compiled SPMD kernel on all 8 cores: a batch of
512 local FFT_4096 per core. Each FFT_4096 = radix-32 stage fused with its
inter-stage transpose as ONE dense matmul per 2-signal slot — the
contraction dim packs (2 signals x 2 planes x 32 j2) so the complex 2x2
structure lives inside a dense [[Wr,Wi],[-Wi,Wr]] weight block (single
accumulation pass; bf16 moving has no minimum-width penalty, unlike f32r) —
then a radix-128 stage whose twiddle exp(-2pi i j1 kap2/4096) is folded
into 32 per-kap2 weight matrices {Br, Bi, -Bi}.

All wire traffic (inputs, weights, outputs) is bfloat16: the kernel is
DMA-bound and bf16 halves HBM bytes while the PE runs bf16 matmuls at the
same 1 column/cycle as f32r. Host-side marshalling lays every DRAM tensor
out so each DMA moves >=2KB contiguous runs per partition (large
descriptors). PSUM (fp32) evacuations are load-balanced across DVE and
Activation (GPSIMD cannot access PSUM). Stage order A0,A1,A2,B0,A3,B1
relaxes every DMA deadline; warmup matmuls pre-ramp the PE p-state; the
first kps after the A3|B1 boundary split their matmuls by c2-half so the
PE bridges the final evacuation latency. Both stages alternate psum
allocations across the two pools (the other stage's pool idles), doubling
the in-flight bank depth and absorbing evacuation-latency jitter.

Local FFT_4096 digits: f = j1 + 128*j2 (j1 in [0,128) fast, j2 in [0,32));
k = kap2 + 32*kap1. Batch b = 128*t + 2*slot + g2 (t chunk of 128, slot in
[0,64), g2 the K-pack pair index). Host does all layout marshalling (numpy
index shuffles); device sees only contiguous [128, X] DMAs.
"""
import numpy as np
import ml_dtypes

import concourse.mybir as mybir
import concourse.tile as tile
from concourse import bacc
from concourse.bass_utils import run_bass_kernel_spmd

NG = 4096                 # global matrix dimension; N = NG*NG
N = NG * NG
NCORES = 8
BPC = NG // NCORES        # 512 signals per core per launch
NCHUNK = 4                # chunks of 128 signals
import os as _os
N_WARMUP = int(_os.environ.get("N_WARMUP", "8"))

_F32 = mybir.dt.float32
_BF16 = mybir.dt.bfloat16
_NPBF16 = ml_dtypes.bfloat16

# ---------------------------------------------------------------------------
# constants (host-side numpy)
# ---------------------------------------------------------------------------

_consts_cache = None


def _make_consts():
    """bdc: [128, 512] bf16  (p = 32g+j2; cols = 256*srcpl + 128*pl' + 32g+kap2)
    bm:  [128, 32, 3, 128] bf16 (p = j1; kap2, {Br,Bi,-Bi}, kap1)"""
    global _consts_cache
    if _consts_cache is not None:
        return _consts_cache
    j2 = np.arange(32)
    W32 = np.exp(-2j * np.pi * np.outer(j2, j2) / 32)
    # dense complex-packed block over (pl, j2) x (pl', kap2); block-diag
    # over g2 (2 signals share the contraction dim)
    blk = np.block([[W32.real, W32.imag], [-W32.imag, W32.real]])  # [64, 64]
    bdc = np.kron(np.eye(2), blk).astype(_NPBF16)                  # [128, 128]

    j1 = np.arange(128)
    W128 = np.exp(-2j * np.pi * np.outer(j1, j1) / 128)
    bm = np.zeros((128, 32, 3, 128), np.float32)
    for kap2 in range(32):
        B = np.exp(-2j * np.pi * j1 * kap2 / 4096)[:, None] * W128  # [j1][kap1]
        bm[:, kap2, 0] = B.real
        bm[:, kap2, 1] = B.imag
        bm[:, kap2, 2] = -B.imag
    bm = bm.astype(_NPBF16)
    _consts_cache = (np.ascontiguousarray(bdc), np.ascontiguousarray(bm))
    return _consts_cache


_tw_cache = None


def _global_twiddle():
    """exp(-2pi i k2g*j1g / N) as complex64 [NG, NG] (k2g rows)."""
    global _tw_cache
    if _tw_cache is None:
        k = np.arange(NG, dtype=np.float64)
        phase = np.outer(k, k) * (-2.0 * np.pi / N)
        _tw_cache = np.exp(1j * phase).astype(np.complex64)
    return _tw_cache


# ---------------------------------------------------------------------------
# marshalling (host)
# ---------------------------------------------------------------------------

def _marshal_in(Vre, Vim):
    """Vre/Vim: [4096 f][512 b] f32 planes -> in2 [4,4,128,16,128] bf16
    (t, q, p=64*g2+32*pl+j2, sl16, j1); signal b = 128t + 2*(16q+sl16) + g2."""
    # [pl, j2, j1, t, sl, g2]
    C = np.stack([V.reshape(32, 128, 4, 64, 2) for V in (Vre, Vim)], axis=0)
    out = (
        C.transpose(3, 4, 5, 0, 1, 2)           # t, sl, g2, pl, j2, j1
        .reshape(NCHUNK, 4, 16, 2, 2, 32, 128)  # t, q, sl16, g2, pl, j2, j1
        .transpose(0, 1, 3, 4, 5, 2, 6)         # t, q, g2, pl, j2, sl16, j1
        .reshape(NCHUNK, 4, 128, 16, 128)
        .astype(_NPBF16)
    )
    return np.ascontiguousarray(out)


def _unmarshal_out(O):
    """out2 [2,16,128,2,512] bf16 (sc, kp, kap1, pl, n2; n2 = 256u + nb,
    nb = 128c2+8bk+2sl+g2) -> (Fre, Fim) planes [4096 k][512 b] f32.
    b = 256*sc + nb ; k = 32*kap1 + 2*kp + u."""
    O8 = O.reshape(2, 16, 128, 2, 2, 256)       # sc, kp, kap1, pl, u, nb
    P = np.ascontiguousarray(
        O8.transpose(3, 2, 1, 4, 0, 5)          # pl, kap1, kp, u, sc, nb
    ).reshape(2, NG, BPC).astype(np.float32)
    return P[0], P[1]


# ---------------------------------------------------------------------------
# device kernel (Bass/Tile), shared by both launches
# ---------------------------------------------------------------------------

_nc_cache = None


def _build_nc():
    global _nc_cache
    if _nc_cache is not None:
        return _nc_cache

    nc = bacc.Bacc(trn_type="TRN2")
    # in layout: [t, q, p = 64g2+32pl+j2, (sl16, j1)]
    in_d = nc.dram_tensor("in2", [NCHUNK, 4, 128, 16, 128], _BF16,
                          kind="ExternalInput")
    bdc_d = nc.dram_tensor("bdc", [128, 128], _BF16, kind="ExternalInput")
    bm_d = nc.dram_tensor("bm", [128, 32, 3, 128], _BF16, kind="ExternalInput")
    # out layout: [superchunk, kap2pair, kap1, pl, n2], n2 = 256u+128c2+4s+g
    out_d = nc.dram_tensor("out2", [NCHUNK // 2, 16, 128, 2, 512], _BF16,
                           kind="ExternalOutput")

    with tile.TileContext(nc) as tc:
        with (
            tc.tile_pool(name="consts", bufs=1) as cpool,
            tc.tile_pool(name="inp", bufs=6) as inpool,
            tc.tile_pool(name="inh", bufs=8) as inhpool,
            tc.tile_pool(name="tp", bufs=2) as tpool,
            tc.tile_pool(name="outp", bufs=16) as outpool,
            tc.tile_pool(name="scr", bufs=4) as scrpool,
            tc.tile_pool(name="pA", bufs=4, space="PSUM") as pA,
            tc.tile_pool(name="pB", bufs=4, space="PSUM") as pB,
        ):
            # --- greedy balancing of PSUM evacuations over DVE + ACT -------
            # (GPSIMD/Pool cannot access PSUM on TRN2 — BIR verifier)
            eng_load = {"v": 0.0, "s": 0.0}
            # rough ns per [128, cols] op: per-elem rate + fixed overhead
            eng_cost = {
                "v": lambda cols: cols * 1.042 + 190.0,   # DVE @0.96GHz
                "s": lambda cols: cols * 0.833 + 230.0,   # ACT @1.2GHz
            }

            def _pick(cols):
                e = min(eng_load, key=lambda k: eng_load[k] + eng_cost[k](cols))
                eng_load[e] += eng_cost[e](cols)
                return e

            def evac(out_ap, in_ap, cols, fast=False, allow=None):
                if allow:
                    e = min(allow, key=lambda k: eng_load[k] + eng_cost[k](cols))
                    eng_load[e] += eng_cost[e](cols)
                else:
                    e = _pick(cols)
                if e == "v":
                    nc.vector.tensor_copy(out_ap, in_ap)
                else:
                    nc.scalar.copy(out_ap, in_ap)

            # --- resident constants ---------------------------------------
            bdc_t = cpool.tile([128, 128], _BF16, tag="bdc")
            bm_t = cpool.tile([128, 32, 3, 128], _BF16, tag="bm")

            in_tiles = {}

            def load_input(t, q, split=1):
                """split=2 halves the DMA (earlier first matmul at startup)."""
                if split == 1:
                    it = inpool.tile([128, 16, 128], _BF16, tag="in")
                    nc.sync.dma_start(it[:], in_d[t, q])
                    in_tiles[(t, q)] = [it]
                else:
                    tiles = []
                    for hh in range(2):
                        it = inhpool.tile([128, 8, 128], _BF16, tag="inh")
                        nc.sync.dma_start(it[:], in_d[t, q, :, 8 * hh:8 * hh + 8])
                        tiles.append(it)
                    in_tiles[(t, q)] = tiles

            def slice_in(t, sl):
                """stationary [128, 128] slice (both planes, 2 signals in K)
                for slot sl within chunk t."""
                tiles = in_tiles[(t, sl // 16)]
                r = sl % 16
                it = tiles[0] if len(tiles) == 1 or r < 8 else tiles[1]
                r = r if len(tiles) == 1 else r % 8
                return it[:, r]

            # PE warmup: dependency-free matmuls on a memset tile keep the
            # tensor engine continuously busy from t~0.3us, so the p-state
            # ramp (half clock for the first 3us of busy) completes before
            # the first real matmul instead of slowing it down.
            wz = scrpool.tile([128, 512], _BF16, tag="warm")
            nc.vector.memset(wz[:], 0.0)
            wbank = pA.tile([128, 512], _F32, tag="psA")
            for _ in range(N_WARMUP):
                nc.tensor.matmul(wbank[:], wz[:, 0:128], wz[:],
                                 start=True, stop=True)

            # DMA order: bdc first (needed by the first matmul), then input
            # chunks 0-2; the three bm pieces ride between the chunk-2
            # quarters (the A0,A1,A2,B0,A3,B1 stage order gives them slack).
            load_input(0, 0, split=2)
            load_input(0, 1, split=2)
            nc.sync.dma_start(bdc_t[:], bdc_d.ap())
            for q in range(2, 4):
                load_input(0, q, split=2)
            for q in range(4):
                load_input(1, q)
            for q in range(4):
                load_input(2, q)
                if q == 1:
                    nc.sync.dma_start(bm_t[:, 0:8], bm_d.ap()[:, 0:8])
            nc.sync.dma_start(bm_t[:, 8:16], bm_d.ap()[:, 8:16])
            nc.sync.dma_start(bm_t[:, 16:32], bm_d.ap()[:, 16:32])
            for q in range(4):
                load_input(3, q)

            tts = []
            for sc in range(NCHUNK // 2):
                # free dims: (c2, bk16, sl4, g2, pl, kap2)
                tt = tpool.tile([128, 2, 16, 4, 2, 2, 32], _BF16, tag="tt")
                tts.append((tt, tt.rearrange("p a b c d e f -> p (a b c d e f)")))

            def stage_a(t):
                # ---- fused stage A + transpose: per 2-signal slot, ONE
                # dense matmul (planes live in the contraction dim):
                #   psum[j1, (g2, pl', kap2)] = S_slot.T @ bdc
                tt, ttf = tts[t // 2]
                c2 = t % 2
                for bk in range(16):             # banks of 4 slots (8 signals)
                    pool_a = pA if bk % 2 == 0 else pB
                    bank = pool_a.tile([128, 512], _F32,
                                       tag="psA" if bk % 2 == 0 else "psB")
                    for e in range(4):
                        sl = 4 * bk + e          # slot within chunk
                        sdat = slice_in(t, sl)
                        ys = slice(128 * e, 128 * e + 128)
                        nc.tensor.matmul(bank[:, ys], sdat, bdc_t[:],
                                         start=True, stop=True)
                    off = (c2 * 16 + bk) * 512
                    if t == 3 and bk >= 8:
                        # the last banks gate stage B (it reads all of tt):
                        # strict bank-level alternation keeps both queues
                        # short so the final evac lands promptly
                        if bk % 2 == 0:
                            nc.vector.tensor_copy(ttf[:, off:off + 512], bank[:])
                            eng_load["v"] += eng_cost["v"](512)
                        else:
                            nc.scalar.copy(ttf[:, off:off + 512], bank[:])
                            eng_load["s"] += eng_cost["s"](512)
                    else:
                        evac(ttf[:, off:off + 512], bank[:], 512)

            def stage_b(sc):
                # ---- stage B: radix-128, per-kap2 twiddled weights, N=256
                tt, ttf = tts[sc]
                for kp in range(16):             # kap2 pairs
                    tail = (sc == NCHUNK // 2 - 1) and kp == 15
                    kp_allow = None
                    if tail:
                        # progressive split of the final kp: each sub-group's
                        # evac+DMA drains while the next sub-group's matmuls
                        # run, so only a tiny store chain trails the last
                        # matmul
                        ot = outpool.tile([128, 2, 512], _BF16, tag="out")
                        for u in range(2):
                            kap2 = 2 * kp + u
                            us = slice(256 * u, 256 * u + 256)
                            yru = pA.tile([128, 256], _F32, tag="psA")
                            yiu = pA.tile([128, 256], _F32, tag="psA")
                            trs = tt[:, :, :, :, :, 0, kap2]
                            tis = tt[:, :, :, :, :, 1, kap2]
                            br = bm_t[:, kap2, 0]
                            bi = bm_t[:, kap2, 1]
                            bni = bm_t[:, kap2, 2]
                            nc.tensor.matmul(yru[:], br, trs, start=True, stop=False)
                            nc.tensor.matmul(yiu[:], br, tis, start=True, stop=False)
                            nc.tensor.matmul(yru[:], bni, tis, start=False, stop=True)
                            nc.tensor.matmul(yiu[:], bi, trs, start=False, stop=True)
                            nc.vector.tensor_copy(ot[:, 0, us], yru[:])
                            nc.scalar.copy(ot[:, 1, us], yiu[:])
                        nc.sync.dma_start(out_d[sc, kp], ot[:])
                        continue
                    # the first kps after the A3|B1 boundary borrow the idle
                    # pA banks; elsewhere alternate pools (pA idles during
                    # stage B) to double the kp pipeline depth
                    boundary = sc == 1 and kp < 2
                    pool = pA if (boundary or kp % 2 == 1) else pB
                    yr = pool.tile([128, 512], _F32, tag="psA" if pool is pA else "psB")
                    yi = pool.tile([128, 512], _F32, tag="psA" if pool is pA else "psB")
                    if boundary:
                        # c2-split matmul order: the c2=0 half of tt(sc1) was
                        # written by A2 (long done), so these matmuls bridge
                        # the bubble while A3's last evacuations land
                        for c2v in range(2):
                            for u in range(2):
                                kap2 = 2 * kp + u
                                ys = slice(256 * u + 128 * c2v,
                                           256 * u + 128 * c2v + 128)
                                trs = tt[:, c2v, :, :, :, 0, kap2]
                                tis = tt[:, c2v, :, :, :, 1, kap2]
                                br = bm_t[:, kap2, 0]
                                bi = bm_t[:, kap2, 1]
                                bni = bm_t[:, kap2, 2]
                                nc.tensor.matmul(yr[:, ys], br, trs,
                                                 start=True, stop=False)
                                nc.tensor.matmul(yi[:, ys], br, tis,
                                                 start=True, stop=False)
                                nc.tensor.matmul(yr[:, ys], bni, tis,
                                                 start=False, stop=True)
                                nc.tensor.matmul(yi[:, ys], bi, trs,
                                                 start=False, stop=True)
                    else:
                        for u in range(2):
                            kap2 = 2 * kp + u
                            ys = slice(256 * u, 256 * u + 256)
                            trs = tt[:, :, :, :, :, 0, kap2]
                            tis = tt[:, :, :, :, :, 1, kap2]
                            br = bm_t[:, kap2, 0]
                            bi = bm_t[:, kap2, 1]
                            bni = bm_t[:, kap2, 2]
                            nc.tensor.matmul(yr[:, ys], br, trs, start=True, stop=False)
                            nc.tensor.matmul(yi[:, ys], br, tis, start=True, stop=False)
                            nc.tensor.matmul(yr[:, ys], bni, tis, start=False, stop=True)
                            nc.tensor.matmul(yi[:, ys], bi, trs, start=False, stop=True)
                    # evac + store
                    ot = outpool.tile([128, 2, 512], _BF16, tag="out")
                    evac(ot[:, 0], yr[:], 512, allow=kp_allow)
                    evac(ot[:, 1], yi[:], 512, allow=kp_allow)
                    nc.sync.dma_start(out_d[sc, kp], ot[:])

            # A2 runs before B0: every weight/input DMA deadline gains the
            # width of one stage-A chunk, removing the early supply crunch
            stage_a(0)
            stage_a(1)
            stage_a(2)
            stage_b(0)
            stage_a(3)
            stage_b(1)

    nc.finalize()
    _nc_cache = nc
    return nc


# ---------------------------------------------------------------------------
# launch helper
# ---------------------------------------------------------------------------

_last_exec_ns = None


def last_exec_time_ns():
    """Sum of HW exec times (ns) of the launches in the last kernel() call,
    when KERNEL_TRACE=1 was set and NTFF profiling is available. None otherwise."""
    return _last_exec_ns


def predicted_exec_time_ns():
    """Cost-model (TimelineSim) predicted HW exec time for both launches, ns."""
    from concourse.timeline_sim import TimelineSim
    nc = _build_nc()
    return int(2 * TimelineSim(nc).simulate())


def _run_launch(cols_re, cols_im):
    """cols_re/cols_im: list of 8 planes [4096 f][512 b] f32.
    Returns list of 8 (Fre, Fim) planes [4096 k][512 b]."""
    global _last_exec_ns
    import os
    nc = _build_nc()
    bdc, bm = _make_consts()
    in_maps = []
    for c in range(NCORES):
        in_maps.append({
            "in2": _marshal_in(cols_re[c], cols_im[c]),
            "bdc": bdc, "bm": bm,
        })
    trace = bool(os.environ.get("KERNEL_TRACE"))
    try:
        res = run_bass_kernel_spmd(nc, in_maps, core_ids=list(range(NCORES)),
                                   trace=trace)
    except ModuleNotFoundError:
        # NTFF profiling hook unavailable under this axon client; run untraced.
        res = run_bass_kernel_spmd(nc, in_maps, core_ids=list(range(NCORES)))
    if trace and getattr(res, "exec_time_ns", None) is not None:
        _last_exec_ns = (_last_exec_ns or 0) + res.exec_time_ns
    return [_unmarshal_out(np.asarray(res.results[c]["out2"]))
            for c in range(NCORES)]


# ---------------------------------------------------------------------------
# public entry point
# ---------------------------------------------------------------------------

def kernel(x: np.ndarray) -> np.ndarray:
    """x: [N, 2] float32 (re, im). Returns FFT(x) as [N, 2] float32."""
    global _last_exec_ns
    _last_exec_ns = None
    x = np.asarray(x)
    Are = np.ascontiguousarray(x[:, 0].reshape(NG, NG))  # [j2g][j1g]
    Aim = np.ascontiguousarray(x[:, 1].reshape(NG, NG))

    # launch 1: FFT over rows (j2g) for each column j1g
    cols_re = [np.ascontiguousarray(Are[:, BPC * c:BPC * (c + 1)]) for c in range(NCORES)]
    cols_im = [np.ascontiguousarray(Aim[:, BPC * c:BPC * (c + 1)]) for c in range(NCORES)]
    l1 = _run_launch(cols_re, cols_im)

    # host: assemble F [k2g][j1g], twiddle, transpose-exchange
    F = np.empty((NG, NG), np.complex64)
    for c in range(NCORES):
        fre, fim = l1[c]
        F[:, BPC * c:BPC * (c + 1)] = fre + 1j * fim
    F *= _global_twiddle()

    # launch 2: FFT over j1g for each row k2g; core d gets rows [512d, 512(d+1))
    cols_re2 = []
    cols_im2 = []
    for d in range(NCORES):
        block = F[BPC * d:BPC * (d + 1), :].T      # [j1g][k2g-local]
        cols_re2.append(np.ascontiguousarray(block.real))
        cols_im2.append(np.ascontiguousarray(block.imag))
    l2 = _run_launch(cols_re2, cols_im2)

    # assemble Xmat [k1g][k2g]; out flat index k = 4096*k1g + k2g
    out = np.empty((NG, NG, 2), np.float32)
    for d in range(NCORES):
        rre, rim = l2[d]
        out[:, BPC * d:BPC * (d + 1), 0] = rre
        out[:, BPC * d:BPC * (d + 1), 1] = rim
    return out.reshape(N, 2)


# revision 131
# speedup vs baseline: 1.0709x; 1.0070x over previous
"""Distributed FFT (N = 2^24 complex points) on 8 Trainium2 NeuronCores.

Four-step (Cooley-Tukey) decomposition N = 4096 x 4096:
  launch 1: per global column j1g, FFT_4096 over j2g      (batch parallel over j1g)
  host:     global twiddle wN^{j1g*k2g} + transpose exchange
  launch 2: per global row k2g, FFT_4096 over j1g         (batch parallel over k2g)

Both launches run the SAME compiled SPMD kernel on all 8 cores: a batch of
512 local FFT_4096 per core. Each FFT_4096 = radix-32 stage fused with its
inter-stage transpose as ONE dense matmul per 2-signal slot — the
contraction dim packs (2 signals x 2 planes x 32 j2) so the complex 2x2
structure lives inside a dense [[Wr,Wi],[-Wi,Wr]] weight block (single
accumulation pass; bf16 moving has no minimum-width penalty, unlike f32r) —
then a radix-128 stage whose twiddle exp(-2pi i j1 kap2/4096) is folded
into 32 per-kap2 weight matrices {Br, Bi, -Bi}.

All wire traffic (inputs, weights, outputs) is bfloat16: the kernel is
DMA-bound and bf16 halves HBM bytes while the PE runs bf16 matmuls at the
same 1 column/cycle as f32r. Host-side marshalling lays every DRAM tensor
out so each DMA moves >=2KB contiguous runs per partition (large
descriptors). PSUM (fp32) evacuations are load-balanced across DVE and
Activation (GPSIMD cannot access PSUM). Stage order A0,A1,A2,B0,A3,B1
relaxes every DMA deadline; warmup matmuls pre-ramp the PE p-state; the
first kps after the A3|B1 boundary split their matmuls by c2-half so the
PE bridges the final evacuation latency. Both stages alternate psum
allocations across the two pools (the other stage's pool idles), doubling
the in-flight bank depth and absorbing evacuation-latency jitter.

Local FFT_4096 digits: f = j1 + 128*j2 (j1 in [0,128) fast, j2 in [0,32));
k = kap2 + 32*kap1. Batch b = 128*t + 2*slot + g2 (t chunk of 128, slot in
[0,64), g2 the K-pack pair index). Host does all layout marshalling (numpy
index shuffles); device sees only contiguous [128, X] DMAs.
"""
import numpy as np
import ml_dtypes

import concourse.mybir as mybir
import concourse.tile as tile
from concourse import bacc
from concourse.bass_utils import run_bass_kernel_spmd

NG = 4096                 # global matrix dimension; N = NG*NG
N = NG * NG
NCORES = 8
BPC = NG // NCORES        # 512 signals per core per launch
NCHUNK = 4                # chunks of 128 signals
import os as _os
N_WARMUP = int(_os.environ.get("N_WARMUP", "8"))

_F32 = mybir.dt.float32
_BF16 = mybir.dt.bfloat16
_NPBF16 = ml_dtypes.bfloat16

# ---------------------------------------------------------------------------
# constants (host-side numpy)
# ---------------------------------------------------------------------------

_consts_cache = None


def _make_consts():
    """bdc: [128, 512] bf16  (p = 32g+j2; cols = 256*srcpl + 128*pl' + 32g+kap2)
    bm:  [128, 32, 3, 128] bf16 (p = j1; kap2, {Br,Bi,-Bi}, kap1)"""
    global _consts_cache
    if _consts_cache is not None:
        return _consts_cache
    j2 = np.arange(32)
    W32 = np.exp(-2j * np.pi * np.outer(j2, j2) / 32)
    # dense complex-packed block over (pl, j2) x (pl', kap2); block-diag
    # over g2 (2 signals share the contraction dim)
    blk = np.block([[W32.real, W32.imag], [-W32.imag, W32.real]])  # [64, 64]
    bdc = np.kron(np.eye(2), blk).astype(_NPBF16)                  # [128, 128]

    j1 = np.arange(128)
    W128 = np.exp(-2j * np.pi * np.outer(j1, j1) / 128)
    bm = np.zeros((128, 32, 3, 128), np.float32)
    for kap2 in range(32):
        B = np.exp(-2j * np.pi * j1 * kap2 / 4096)[:, None] * W128  # [j1][kap1]
        bm[:, kap2, 0] = B.real
        bm[:, kap2, 1] = B.imag
        bm[:, kap2, 2] = -B.imag
    bm = bm.astype(_NPBF16)
    _consts_cache = (np.ascontiguousarray(bdc), np.ascontiguousarray(bm))
    return _consts_cache


_tw_cache = None


def _global_twiddle():
    """exp(-2pi i k2g*j1g / N) as complex64 [NG, NG] (k2g rows)."""
    global _tw_cache
    if _tw_cache is None:
        k = np.arange(NG, dtype=np.float64)
        phase = np.outer(k, k) * (-2.0 * np.pi / N)
        _tw_cache = np.exp(1j * phase).astype(np.complex64)
    return _tw_cache


# ---------------------------------------------------------------------------
# marshalling (host)
# ---------------------------------------------------------------------------

def _marshal_in(Vre, Vim):
    """Vre/Vim: [4096 f][512 b] f32 planes -> in2 [4,4,128,16,128] bf16
    (t, q, p=64*g2+32*pl+j2, sl16, j1); signal b = 128t + 2*(16q+sl16) + g2."""
    # [pl, j2, j1, t, sl, g2]
    C = np.stack([V.reshape(32, 128, 4, 64, 2) for V in (Vre, Vim)], axis=0)
    out = (
        C.transpose(3, 4, 5, 0, 1, 2)           # t, sl, g2, pl, j2, j1
        .reshape(NCHUNK, 4, 16, 2, 2, 32, 128)  # t, q, sl16, g2, pl, j2, j1
        .transpose(0, 1, 3, 4, 5, 2, 6)         # t, q, g2, pl, j2, sl16, j1
        .reshape(NCHUNK, 4, 128, 16, 128)
        .astype(_NPBF16)
    )
    return np.ascontiguousarray(out)


def _unmarshal_out(O):
    """out2 [2,16,128,2,512] bf16 (sc, kp, kap1, pl, n2; n2 = 256u + nb,
    nb = 128c2+8bk+2sl+g2) -> (Fre, Fim) planes [4096 k][512 b] f32.
    b = 256*sc + nb ; k = 32*kap1 + 2*kp + u."""
    O8 = O.reshape(2, 16, 128, 2, 2, 256)       # sc, kp, kap1, pl, u, nb
    P = np.ascontiguousarray(
        O8.transpose(3, 2, 1, 4, 0, 5)          # pl, kap1, kp, u, sc, nb
    ).reshape(2, NG, BPC).astype(np.float32)
    return P[0], P[1]


# ---------------------------------------------------------------------------
# device kernel (Bass/Tile), shared by both launches
# ---------------------------------------------------------------------------

_nc_cache = None


def _build_nc():
    global _nc_cache
    if _nc_cache is not None:
        return _nc_cache

    nc = bacc.Bacc(trn_type="TRN2")
    # in layout: [t, q, p = 64g2+32pl+j2, (sl16, j1)]
    in_d = nc.dram_tensor("in2", [NCHUNK, 4, 128, 16, 128], _BF16,
                          kind="ExternalInput")
    bdc_d = nc.dram_tensor("bdc", [128, 128], _BF16, kind="ExternalInput")
    bm_d = nc.dram_tensor("bm", [128, 32, 3, 128], _BF16, kind="ExternalInput")
    # out layout: [superchunk, kap2pair, kap1, pl, n2], n2 = 256u+128c2+4s+g
    out_d = nc.dram_tensor("out2", [NCHUNK // 2, 16, 128, 2, 512], _BF16,
                           kind="ExternalOutput")

    with tile.TileContext(nc) as tc:
        with (
            tc.tile_pool(name="consts", bufs=1) as cpool,
            tc.tile_pool(name="inp", bufs=6) as inpool,
            tc.tile_pool(name="inh", bufs=8) as inhpool,
            tc.tile_pool(name="tp", bufs=2) as tpool,
            tc.tile_pool(name="outp", bufs=16) as outpool,
            tc.tile_pool(name="scr", bufs=4) as scrpool,
            tc.tile_pool(name="pA", bufs=4, space="PSUM") as pA,
            tc.tile_pool(name="pB", bufs=4, space="PSUM") as pB,
        ):
            # --- greedy balancing of PSUM evacuations over DVE + ACT -------
            # (GPSIMD/Pool cannot access PSUM on TRN2 — BIR verifier)
            eng_load = {"v": 0.0, "s": 0.0}
            # rough ns per [128, cols] op: per-elem rate + fixed overhead
            eng_cost = {
                "v": lambda cols: cols * 1.042 + 190.0,   # DVE @0.96GHz
                "s": lambda cols: cols * 0.833 + 230.0,   # ACT @1.2GHz
            }

            def _pick(cols):
                e = min(eng_load, key=lambda k: eng_load[k] + eng_cost[k](cols))
                eng_load[e] += eng_cost[e](cols)
                return e

            def evac(out_ap, in_ap, cols, fast=False, allow=None):
                if allow:
                    e = min(allow, key=lambda k: eng_load[k] + eng_cost[k](cols))
                    eng_load[e] += eng_cost[e](cols)
                else:
                    e = _pick(cols)
                if e == "v":
                    nc.vector.tensor_copy(out_ap, in_ap)
                else:
                    nc.scalar.copy(out_ap, in_ap)

            # --- resident constants ---------------------------------------
            bdc_t = cpool.tile([128, 128], _BF16, tag="bdc")
            bm_t = cpool.tile([128, 32, 3, 128], _BF16, tag="bm")

            in_tiles = {}

            def load_input(t, q, split=1):
                """split=2 halves the DMA (earlier first matmul at startup)."""
                if split == 1:
                    it = inpool.tile([128, 16, 128], _BF16, tag="in")
                    nc.sync.dma_start(it[:], in_d[t, q])
                    in_tiles[(t, q)] = [it]
                else:
                    tiles = []
                    for hh in range(2):
                        it = inhpool.tile([128, 8, 128], _BF16, tag="inh")
                        nc.sync.dma_start(it[:], in_d[t, q, :, 8 * hh:8 * hh + 8])
                        tiles.append(it)
                    in_tiles[(t, q)] = tiles

            def slice_in(t, sl):
                """stationary [128, 128] slice (both planes, 2 signals in K)
                for slot sl within chunk t."""
                tiles = in_tiles[(t, sl // 16)]
                r = sl % 16
                it = tiles[0] if len(tiles) == 1 or r < 8 else tiles[1]
                r = r if len(tiles) == 1 else r % 8
                return it[:, r]

            # PE warmup: dependency-free matmuls on a memset tile keep the
            # tensor engine continuously busy from t~0.3us, so the p-state
            # ramp (half clock for the first 3us of busy) completes before
            # the first real matmul instead of slowing it down.
            wz = scrpool.tile([128, 512], _BF16, tag="warm")
            nc.vector.memset(wz[:], 0.0)
            wbank = pA.tile([128, 512], _F32, tag="psA")
            for _ in range(N_WARMUP):
                nc.tensor.matmul(wbank[:], wz[:, 0:128], wz[:],
                                 start=True, stop=True)

            # DMA order: bdc first (needed by the first matmul), then input
            # chunks 0-2; the three bm pieces ride between the chunk-2
            # quarters (the A0,A1,A2,B0,A3,B1 stage order gives them slack).
            load_input(0, 0, split=2)
            load_input(0, 1, split=2)
            nc.sync.dma_start(bdc_t[:], bdc_d.ap())
            for q in range(2, 4):
                load_input(0, q, split=2)
            for q in range(4):
                load_input(1, q)
            for q in range(4):
                load_input(2, q, split=2)
                if q == 1:
                    nc.sync.dma_start(bm_t[:, 0:8], bm_d.ap()[:, 0:8])
            nc.sync.dma_start(bm_t[:, 8:16], bm_d.ap()[:, 8:16])
            nc.sync.dma_start(bm_t[:, 16:32], bm_d.ap()[:, 16:32])
            for q in range(4):
                load_input(3, q, split=2)

            tts = []
            for sc in range(NCHUNK // 2):
                # free dims: (c2, bk16, sl4, g2, pl, kap2)
                tt = tpool.tile([128, 2, 16, 4, 2, 2, 32], _BF16, tag="tt")
                tts.append((tt, tt.rearrange("p a b c d e f -> p (a b c d e f)")))

            def stage_a(t):
                # ---- fused stage A + transpose: per 2-signal slot, ONE
                # dense matmul (planes live in the contraction dim):
                #   psum[j1, (g2, pl', kap2)] = S_slot.T @ bdc
                tt, ttf = tts[t // 2]
                c2 = t % 2
                for bk in range(16):             # banks of 4 slots (8 signals)
                    pool_a = pA if bk % 2 == 0 else pB
                    bank = pool_a.tile([128, 512], _F32,
                                       tag="psA" if bk % 2 == 0 else "psB")
                    for e in range(4):
                        sl = 4 * bk + e          # slot within chunk
                        sdat = slice_in(t, sl)
                        ys = slice(128 * e, 128 * e + 128)
                        nc.tensor.matmul(bank[:, ys], sdat, bdc_t[:],
                                         start=True, stop=True)
                    off = (c2 * 16 + bk) * 512
                    if t == 3 and bk >= 8:
                        # the last banks gate stage B (it reads all of tt):
                        # strict bank-level alternation keeps both queues
                        # short so the final evac lands promptly
                        if bk % 2 == 0:
                            nc.vector.tensor_copy(ttf[:, off:off + 512], bank[:])
                            eng_load["v"] += eng_cost["v"](512)
                        else:
                            nc.scalar.copy(ttf[:, off:off + 512], bank[:])
                            eng_load["s"] += eng_cost["s"](512)
                    else:
                        evac(ttf[:, off:off + 512], bank[:], 512)

            def stage_b(sc):
                # ---- stage B: radix-128, per-kap2 twiddled weights, N=256
                tt, ttf = tts[sc]
                for kp in range(16):             # kap2 pairs
                    tail = (sc == NCHUNK // 2 - 1) and kp == 15
                    kp_allow = None
                    if tail:
                        # progressive split of the final kp: each sub-group's
                        # evac+DMA drains while the next sub-group's matmuls
                        # run, so only a tiny store chain trails the last
                        # matmul
                        ot = outpool.tile([128, 2, 512], _BF16, tag="out")
                        for u in range(2):
                            kap2 = 2 * kp + u
                            us = slice(256 * u, 256 * u + 256)
                            yru = pA.tile([128, 256], _F32, tag="psA")
                            yiu = pA.tile([128, 256], _F32, tag="psA")
                            trs = tt[:, :, :, :, :, 0, kap2]
                            tis = tt[:, :, :, :, :, 1, kap2]
                            br = bm_t[:, kap2, 0]
                            bi = bm_t[:, kap2, 1]
                            bni = bm_t[:, kap2, 2]
                            nc.tensor.matmul(yru[:], br, trs, start=True, stop=False)
                            nc.tensor.matmul(yiu[:], br, tis, start=True, stop=False)
                            nc.tensor.matmul(yru[:], bni, tis, start=False, stop=True)
                            nc.tensor.matmul(yiu[:], bi, trs, start=False, stop=True)
                            nc.vector.tensor_copy(ot[:, 0, us], yru[:])
                            nc.scalar.copy(ot[:, 1, us], yiu[:])
                        nc.sync.dma_start(out_d[sc, kp], ot[:])
                        continue
                    # the first kps after the A3|B1 boundary borrow the idle
                    # pA banks; elsewhere alternate pools (pA idles during
                    # stage B) to double the kp pipeline depth
                    boundary = sc == 1 and kp < 2
                    pool = pA if (boundary or kp % 2 == 1) else pB
                    yr = pool.tile([128, 512], _F32, tag="psA" if pool is pA else "psB")
                    yi = pool.tile([128, 512], _F32, tag="psA" if pool is pA else "psB")
                    if boundary:
                        # c2-split matmul order: the c2=0 half of tt(sc1) was
                        # written by A2 (long done), so these matmuls bridge
                        # the bubble while A3's last evacuations land
                        for c2v in range(2):
                            for u in range(2):
                                kap2 = 2 * kp + u
                                ys = slice(256 * u + 128 * c2v,
                                           256 * u + 128 * c2v + 128)
                                trs = tt[:, c2v, :, :, :, 0, kap2]
                                tis = tt[:, c2v, :, :, :, 1, kap2]
                                br = bm_t[:, kap2, 0]
                                bi = bm_t[:, kap2, 1]
                                bni = bm_t[:, kap2, 2]
                                nc.tensor.matmul(yr[:, ys], br, trs,
                                                 start=True, stop=False)
                                nc.tensor.matmul(yi[:, ys], br, tis,
                                                 start=True, stop=False)
                                nc.tensor.matmul(yr[:, ys], bni, tis,
                                                 start=False, stop=True)
                                nc.tensor.matmul(yi[:, ys], bi, trs,
                                                 start=False, stop=True)
                    else:
                        for u in range(2):
                            kap2 = 2 * kp + u
                            ys = slice(256 * u, 256 * u + 256)
                            trs = tt[:, :, :, :, :, 0, kap2]
                            tis = tt[:, :, :, :, :, 1, kap2]
                            br = bm_t[:, kap2, 0]
                            bi = bm_t[:, kap2, 1]
                            bni = bm_t[:, kap2, 2]
                            nc.tensor.matmul(yr[:, ys], br, trs, start=True, stop=False)
                            nc.tensor.matmul(yi[:, ys], br, tis, start=True, stop=False)
                            nc.tensor.matmul(yr[:, ys], bni, tis, start=False, stop=True)
                            nc.tensor.matmul(yi[:, ys], bi, trs, start=False, stop=True)
                    # evac + store
                    ot = outpool.tile([128, 2, 512], _BF16, tag="out")
                    evac(ot[:, 0], yr[:], 512, allow=kp_allow)
                    evac(ot[:, 1], yi[:], 512, allow=kp_allow)
                    nc.sync.dma_start(out_d[sc, kp], ot[:])

            # A2 runs before B0: every weight/input DMA deadline gains the
            # width of one stage-A chunk, removing the early supply crunch
            stage_a(0)
            stage_a(1)
            stage_a(2)
            stage_b(0)
            stage_a(3)
            stage_b(1)

    nc.finalize()
    _nc_cache = nc
    return nc


# ---------------------------------------------------------------------------
# launch helper
# ---------------------------------------------------------------------------

_last_exec_ns = None


def last_exec_time_ns():
    """Sum of HW exec times (ns) of the launches in the last kernel() call,
    when KERNEL_TRACE=1 was set and NTFF profiling is available. None otherwise."""
    return _last_exec_ns


def predicted_exec_time_ns():
    """Cost-model (TimelineSim) predicted HW exec time for both launches, ns."""
    from concourse.timeline_sim import TimelineSim
    nc = _build_nc()
    return int(2 * TimelineSim(nc).simulate())


def _run_launch(cols_re, cols_im):
    """cols_re/cols_im: list of 8 planes [4096 f][512 b] f32.
    Returns list of 8 (Fre, Fim) planes [4096 k][512 b]."""
    global _last_exec_ns
    import os
    nc = _build_nc()
    bdc, bm = _make_consts()
    in_maps = []
    for c in range(NCORES):
        in_maps.append({
            "in2": _marshal_in(cols_re[c], cols_im[c]),
            "bdc": bdc, "bm": bm,
        })
    trace = bool(os.environ.get("KERNEL_TRACE"))
    try:
        res = run_bass_kernel_spmd(nc, in_maps, core_ids=list(range(NCORES)),
                                   trace=trace)
    except ModuleNotFoundError:
        # NTFF profiling hook unavailable under this axon client; run untraced.
        res = run_bass_kernel_spmd(nc, in_maps, core_ids=list(range(NCORES)))
    if trace and getattr(res, "exec_time_ns", None) is not None:
        _last_exec_ns = (_last_exec_ns or 0) + res.exec_time_ns
    return [_unmarshal_out(np.asarray(res.results[c]["out2"]))
            for c in range(NCORES)]


# ---------------------------------------------------------------------------
# public entry point
# ---------------------------------------------------------------------------

def kernel(x: np.ndarray) -> np.ndarray:
    """x: [N, 2] float32 (re, im). Returns FFT(x) as [N, 2] float32."""
    global _last_exec_ns
    _last_exec_ns = None
    x = np.asarray(x)
    Are = np.ascontiguousarray(x[:, 0].reshape(NG, NG))  # [j2g][j1g]
    Aim = np.ascontiguousarray(x[:, 1].reshape(NG, NG))

    # launch 1: FFT over rows (j2g) for each column j1g
    cols_re = [np.ascontiguousarray(Are[:, BPC * c:BPC * (c + 1)]) for c in range(NCORES)]
    cols_im = [np.ascontiguousarray(Aim[:, BPC * c:BPC * (c + 1)]) for c in range(NCORES)]
    l1 = _run_launch(cols_re, cols_im)

    # host: assemble F [k2g][j1g], twiddle, transpose-exchange
    F = np.empty((NG, NG), np.complex64)
    for c in range(NCORES):
        fre, fim = l1[c]
        F[:, BPC * c:BPC * (c + 1)] = fre + 1j * fim
    F *= _global_twiddle()

    # launch 2: FFT over j1g for each row k2g; core d gets rows [512d, 512(d+1))
    cols_re2 = []
    cols_im2 = []
    for d in range(NCORES):
        block = F[BPC * d:BPC * (d + 1), :].T      # [j1g][k2g-local]
        cols_re2.append(np.ascontiguousarray(block.real))
        cols_im2.append(np.ascontiguousarray(block.imag))
    l2 = _run_launch(cols_re2, cols_im2)

    # assemble Xmat [k1g][k2g]; out flat index k = 4096*k1g + k2g
    out = np.empty((NG, NG, 2), np.float32)
    for d in range(NCORES):
        rre, rim = l2[d]
        out[:, BPC * d:BPC * (d + 1), 0] = rre
        out[:, BPC * d:BPC * (d + 1), 1] = rim
    return out.reshape(N, 2)


# revision 134
# speedup vs baseline: 1.0785x; 1.0071x over previous
"""Distributed FFT (N = 2^24 complex points) on 8 Trainium2 NeuronCores.

Four-step (Cooley-Tukey) decomposition N = 4096 x 4096:
  launch 1: per global column j1g, FFT_4096 over j2g      (batch parallel over j1g)
  host:     global twiddle wN^{j1g*k2g} + transpose exchange
  launch 2: per global row k2g, FFT_4096 over j1g         (batch parallel over k2g)

Both launches run the SAME compiled SPMD kernel on all 8 cores: a batch of
512 local FFT_4096 per core. Each FFT_4096 = radix-32 stage fused with its
inter-stage transpose as ONE dense matmul per 2-signal slot — the
contraction dim packs (2 signals x 2 planes x 32 j2) so the complex 2x2
structure lives inside a dense [[Wr,Wi],[-Wi,Wr]] weight block (single
accumulation pass; bf16 moving has no minimum-width penalty, unlike f32r) —
then a radix-128 stage whose twiddle exp(-2pi i j1 kap2/4096) is folded
into 32 per-kap2 weight matrices {Br, Bi, -Bi}.

All wire traffic (inputs, weights, outputs) is bfloat16: the kernel is
DMA-bound and bf16 halves HBM bytes while the PE runs bf16 matmuls at the
same 1 column/cycle as f32r. Host-side marshalling lays every DRAM tensor
out so each DMA moves >=2KB contiguous runs per partition (large
descriptors). PSUM (fp32) evacuations are load-balanced across DVE and
Activation (GPSIMD cannot access PSUM). Stage order A0,A1,A2,B0,A3,B1
relaxes every DMA deadline; warmup matmuls pre-ramp the PE p-state; the
first kps after the A3|B1 boundary split their matmuls by c2-half so the
PE bridges the final evacuation latency. Both stages alternate psum
allocations across the two pools (the other stage's pool idles), doubling
the in-flight bank depth and absorbing evacuation-latency jitter.

Local FFT_4096 digits: f = j1 + 128*j2 (j1 in [0,128) fast, j2 in [0,32));
k = kap2 + 32*kap1. Batch b = 128*t + 2*slot + g2 (t chunk of 128, slot in
[0,64), g2 the K-pack pair index). Host does all layout marshalling (numpy
index shuffles); device sees only contiguous [128, X] DMAs.
"""
import numpy as np
import ml_dtypes

import concourse.mybir as mybir
import concourse.tile as tile
from concourse import bacc
from concourse.bass_utils import run_bass_kernel_spmd

NG = 4096                 # global matrix dimension; N = NG*NG
N = NG * NG
NCORES = 8
BPC = NG // NCORES        # 512 signals per core per launch
NCHUNK = 4                # chunks of 128 signals
import os as _os
N_WARMUP = int(_os.environ.get("N_WARMUP", "8"))

_F32 = mybir.dt.float32
_BF16 = mybir.dt.bfloat16
_NPBF16 = ml_dtypes.bfloat16

# ---------------------------------------------------------------------------
# constants (host-side numpy)
# ---------------------------------------------------------------------------

_consts_cache = None


def _make_consts():
    """bdc: [128, 512] bf16  (p = 32g+j2; cols = 256*srcpl + 128*pl' + 32g+kap2)
    bm:  [128, 32, 3, 128] bf16 (p = j1; kap2, {Br,Bi,-Bi}, kap1)"""
    global _consts_cache
    if _consts_cache is not None:
        return _consts_cache
    j2 = np.arange(32)
    W32 = np.exp(-2j * np.pi * np.outer(j2, j2) / 32)
    # dense complex-packed block over (pl, j2) x (pl', kap2); block-diag
    # over g2 (2 signals share the contraction dim)
    blk = np.block([[W32.real, W32.imag], [-W32.imag, W32.real]])  # [64, 64]
    bdc = np.kron(np.eye(2), blk).astype(_NPBF16)                  # [128, 128]

    j1 = np.arange(128)
    W128 = np.exp(-2j * np.pi * np.outer(j1, j1) / 128)
    bm = np.zeros((128, 32, 3, 128), np.float32)
    for kap2 in range(32):
        B = np.exp(-2j * np.pi * j1 * kap2 / 4096)[:, None] * W128  # [j1][kap1]
        bm[:, kap2, 0] = B.real
        bm[:, kap2, 1] = B.imag
        bm[:, kap2, 2] = -B.imag
    bm = bm.astype(_NPBF16)
    _consts_cache = (np.ascontiguousarray(bdc), np.ascontiguousarray(bm))
    return _consts_cache


_tw_cache = None


def _global_twiddle():
    """exp(-2pi i k2g*j1g / N) as complex64 [NG, NG] (k2g rows)."""
    global _tw_cache
    if _tw_cache is None:
        k = np.arange(NG, dtype=np.float64)
        phase = np.outer(k, k) * (-2.0 * np.pi / N)
        _tw_cache = np.exp(1j * phase).astype(np.complex64)
    return _tw_cache


# ---------------------------------------------------------------------------
# marshalling (host)
# ---------------------------------------------------------------------------

def _marshal_in(Vre, Vim):
    """Vre/Vim: [4096 f][512 b] f32 planes -> in2 [4,4,128,16,128] bf16
    (t, q, p=64*g2+32*pl+j2, sl16, j1); signal b = 128t + 2*(16q+sl16) + g2."""
    # [pl, j2, j1, t, sl, g2]
    C = np.stack([V.reshape(32, 128, 4, 64, 2) for V in (Vre, Vim)], axis=0)
    out = (
        C.transpose(3, 4, 5, 0, 1, 2)           # t, sl, g2, pl, j2, j1
        .reshape(NCHUNK, 4, 16, 2, 2, 32, 128)  # t, q, sl16, g2, pl, j2, j1
        .transpose(0, 1, 3, 4, 5, 2, 6)         # t, q, g2, pl, j2, sl16, j1
        .reshape(NCHUNK, 4, 128, 16, 128)
        .astype(_NPBF16)
    )
    return np.ascontiguousarray(out)


def _unmarshal_out(O):
    """out2 [2,16,128,2,512] bf16 (sc, kp, kap1, pl, n2; n2 = 256u + nb,
    nb = 128c2+8bk+2sl+g2) -> (Fre, Fim) planes [4096 k][512 b] f32.
    b = 256*sc + nb ; k = 32*kap1 + 2*kp + u."""
    O8 = O.reshape(2, 16, 128, 2, 2, 256)       # sc, kp, kap1, pl, u, nb
    P = np.ascontiguousarray(
        O8.transpose(3, 2, 1, 4, 0, 5)          # pl, kap1, kp, u, sc, nb
    ).reshape(2, NG, BPC).astype(np.float32)
    return P[0], P[1]


# ---------------------------------------------------------------------------
# device kernel (Bass/Tile), shared by both launches
# ---------------------------------------------------------------------------

_nc_cache = None


def _build_nc():
    global _nc_cache
    if _nc_cache is not None:
        return _nc_cache

    nc = bacc.Bacc(trn_type="TRN2")
    # in layout: [t, q, p = 64g2+32pl+j2, (sl16, j1)]
    in_d = nc.dram_tensor("in2", [NCHUNK, 4, 128, 16, 128], _BF16,
                          kind="ExternalInput")
    bdc_d = nc.dram_tensor("bdc", [128, 128], _BF16, kind="ExternalInput")
    bm_d = nc.dram_tensor("bm", [128, 32, 3, 128], _BF16, kind="ExternalInput")
    # out layout: [superchunk, kap2pair, kap1, pl, n2], n2 = 256u+128c2+4s+g
    out_d = nc.dram_tensor("out2", [NCHUNK // 2, 16, 128, 2, 512], _BF16,
                           kind="ExternalOutput")

    with tile.TileContext(nc) as tc:
        with (
            tc.tile_pool(name="consts", bufs=1) as cpool,
            tc.tile_pool(name="inp", bufs=6) as inpool,
            tc.tile_pool(name="inh", bufs=8) as inhpool,
            tc.tile_pool(name="tp", bufs=2) as tpool,
            tc.tile_pool(name="outp", bufs=16) as outpool,
            tc.tile_pool(name="scr", bufs=4) as scrpool,
            tc.tile_pool(name="pA", bufs=4, space="PSUM") as pA,
            tc.tile_pool(name="pB", bufs=4, space="PSUM") as pB,
        ):
            # --- greedy balancing of PSUM evacuations over DVE + ACT -------
            # (GPSIMD/Pool cannot access PSUM on TRN2 — BIR verifier)
            eng_load = {"v": 0.0, "s": 0.0}
            # rough ns per [128, cols] op: per-elem rate + fixed overhead
            eng_cost = {
                "v": lambda cols: cols * 1.042 + 190.0,   # DVE @0.96GHz
                "s": lambda cols: cols * 0.833 + 230.0,   # ACT @1.2GHz
            }

            def _pick(cols):
                e = min(eng_load, key=lambda k: eng_load[k] + eng_cost[k](cols))
                eng_load[e] += eng_cost[e](cols)
                return e

            def evac(out_ap, in_ap, cols, fast=False, allow=None):
                if allow:
                    e = min(allow, key=lambda k: eng_load[k] + eng_cost[k](cols))
                    eng_load[e] += eng_cost[e](cols)
                else:
                    e = _pick(cols)
                if e == "v":
                    nc.vector.tensor_copy(out_ap, in_ap)
                else:
                    nc.scalar.copy(out_ap, in_ap)

            # --- resident constants ---------------------------------------
            bdc_t = cpool.tile([128, 128], _BF16, tag="bdc")
            bm_t = cpool.tile([128, 32, 3, 128], _BF16, tag="bm")

            in_tiles = {}

            def load_input(t, q, split=1):
                """split=2 halves the DMA (earlier first matmul at startup)."""
                if split == 1:
                    it = inpool.tile([128, 16, 128], _BF16, tag="in")
                    nc.sync.dma_start(it[:], in_d[t, q])
                    in_tiles[(t, q)] = [it]
                else:
                    tiles = []
                    for hh in range(2):
                        it = inhpool.tile([128, 8, 128], _BF16, tag="inh")
                        nc.sync.dma_start(it[:], in_d[t, q, :, 8 * hh:8 * hh + 8])
                        tiles.append(it)
                    in_tiles[(t, q)] = tiles

            def slice_in(t, sl):
                """stationary [128, 128] slice (both planes, 2 signals in K)
                for slot sl within chunk t."""
                tiles = in_tiles[(t, sl // 16)]
                r = sl % 16
                it = tiles[0] if len(tiles) == 1 or r < 8 else tiles[1]
                r = r if len(tiles) == 1 else r % 8
                return it[:, r]

            # PE warmup: dependency-free matmuls on a memset tile keep the
            # tensor engine continuously busy from t~0.3us, so the p-state
            # ramp (half clock for the first 3us of busy) completes before
            # the first real matmul instead of slowing it down.
            wz = scrpool.tile([128, 512], _BF16, tag="warm")
            nc.vector.memset(wz[:], 0.0)
            wbank = pA.tile([128, 512], _F32, tag="psA")
            for _ in range(N_WARMUP):
                nc.tensor.matmul(wbank[:], wz[:, 0:128], wz[:],
                                 start=True, stop=True)

            # DMA order: bdc first (needed by the first matmul), then input
            # chunks 0-2; the three bm pieces ride between the chunk-2
            # quarters (the A0,A1,A2,B0,A3,B1 stage order gives them slack).
            load_input(0, 0, split=2)
            load_input(0, 1, split=2)
            nc.sync.dma_start(bdc_t[:], bdc_d.ap())
            for q in range(2, 4):
                load_input(0, q, split=2)
            for q in range(4):
                load_input(1, q)
            for q in range(4):
                load_input(2, q, split=2)
                if q == 3:
                    nc.sync.dma_start(bm_t[:, 0:8], bm_d.ap()[:, 0:8])
            nc.sync.dma_start(bm_t[:, 8:16], bm_d.ap()[:, 8:16])
            nc.sync.dma_start(bm_t[:, 16:32], bm_d.ap()[:, 16:32])
            for q in range(4):
                load_input(3, q, split=2)

            tts = []
            for sc in range(NCHUNK // 2):
                # free dims: (c2, bk16, sl4, g2, pl, kap2)
                tt = tpool.tile([128, 2, 16, 4, 2, 2, 32], _BF16, tag="tt")
                tts.append((tt, tt.rearrange("p a b c d e f -> p (a b c d e f)")))

            def stage_a(t):
                # ---- fused stage A + transpose: per 2-signal slot, ONE
                # dense matmul (planes live in the contraction dim):
                #   psum[j1, (g2, pl', kap2)] = S_slot.T @ bdc
                tt, ttf = tts[t // 2]
                c2 = t % 2
                for bk in range(16):             # banks of 4 slots (8 signals)
                    pool_a = pA if bk % 2 == 0 else pB
                    bank = pool_a.tile([128, 512], _F32,
                                       tag="psA" if bk % 2 == 0 else "psB")
                    for e in range(4):
                        sl = 4 * bk + e          # slot within chunk
                        sdat = slice_in(t, sl)
                        ys = slice(128 * e, 128 * e + 128)
                        nc.tensor.matmul(bank[:, ys], sdat, bdc_t[:],
                                         start=True, stop=True)
                    off = (c2 * 16 + bk) * 512
                    if t == 3 and bk >= 8:
                        # the last banks gate stage B (it reads all of tt):
                        # strict bank-level alternation keeps both queues
                        # short so the final evac lands promptly
                        if bk % 2 == 0:
                            nc.vector.tensor_copy(ttf[:, off:off + 512], bank[:])
                            eng_load["v"] += eng_cost["v"](512)
                        else:
                            nc.scalar.copy(ttf[:, off:off + 512], bank[:])
                            eng_load["s"] += eng_cost["s"](512)
                    else:
                        evac(ttf[:, off:off + 512], bank[:], 512)

            def stage_b(sc):
                # ---- stage B: radix-128, per-kap2 twiddled weights, N=256
                tt, ttf = tts[sc]
                for kp in range(16):             # kap2 pairs
                    tail = (sc == NCHUNK // 2 - 1) and kp == 15
                    kp_allow = None
                    if tail:
                        # progressive split of the final kp: each sub-group's
                        # evac+DMA drains while the next sub-group's matmuls
                        # run, so only a tiny store chain trails the last
                        # matmul
                        ot = outpool.tile([128, 2, 512], _BF16, tag="out")
                        for u in range(2):
                            kap2 = 2 * kp + u
                            us = slice(256 * u, 256 * u + 256)
                            yru = pA.tile([128, 256], _F32, tag="psA")
                            yiu = pA.tile([128, 256], _F32, tag="psA")
                            trs = tt[:, :, :, :, :, 0, kap2]
                            tis = tt[:, :, :, :, :, 1, kap2]
                            br = bm_t[:, kap2, 0]
                            bi = bm_t[:, kap2, 1]
                            bni = bm_t[:, kap2, 2]
                            nc.tensor.matmul(yru[:], br, trs, start=True, stop=False)
                            nc.tensor.matmul(yiu[:], br, tis, start=True, stop=False)
                            nc.tensor.matmul(yru[:], bni, tis, start=False, stop=True)
                            nc.tensor.matmul(yiu[:], bi, trs, start=False, stop=True)
                            nc.vector.tensor_copy(ot[:, 0, us], yru[:])
                            nc.scalar.copy(ot[:, 1, us], yiu[:])
                        nc.sync.dma_start(out_d[sc, kp], ot[:])
                        continue
                    # the first kps after the A3|B1 boundary borrow the idle
                    # pA banks; elsewhere alternate pools (pA idles during
                    # stage B) to double the kp pipeline depth
                    boundary = sc == 1 and kp < 2
                    pool = pA if (boundary or kp % 2 == 1) else pB
                    yr = pool.tile([128, 512], _F32, tag="psA" if pool is pA else "psB")
                    yi = pool.tile([128, 512], _F32, tag="psA" if pool is pA else "psB")
                    if boundary:
                        # c2-split matmul order: the c2=0 half of tt(sc1) was
                        # written by A2 (long done), so these matmuls bridge
                        # the bubble while A3's last evacuations land
                        for c2v in range(2):
                            for u in range(2):
                                kap2 = 2 * kp + u
                                ys = slice(256 * u + 128 * c2v,
                                           256 * u + 128 * c2v + 128)
                                trs = tt[:, c2v, :, :, :, 0, kap2]
                                tis = tt[:, c2v, :, :, :, 1, kap2]
                                br = bm_t[:, kap2, 0]
                                bi = bm_t[:, kap2, 1]
                                bni = bm_t[:, kap2, 2]
                                nc.tensor.matmul(yr[:, ys], br, trs,
                                                 start=True, stop=False)
                                nc.tensor.matmul(yi[:, ys], br, tis,
                                                 start=True, stop=False)
                                nc.tensor.matmul(yr[:, ys], bni, tis,
                                                 start=False, stop=True)
                                nc.tensor.matmul(yi[:, ys], bi, trs,
                                                 start=False, stop=True)
                    else:
                        for u in range(2):
                            kap2 = 2 * kp + u
                            ys = slice(256 * u, 256 * u + 256)
                            trs = tt[:, :, :, :, :, 0, kap2]
                            tis = tt[:, :, :, :, :, 1, kap2]
                            br = bm_t[:, kap2, 0]
                            bi = bm_t[:, kap2, 1]
                            bni = bm_t[:, kap2, 2]
                            nc.tensor.matmul(yr[:, ys], br, trs, start=True, stop=False)
                            nc.tensor.matmul(yi[:, ys], br, tis, start=True, stop=False)
                            nc.tensor.matmul(yr[:, ys], bni, tis, start=False, stop=True)
                            nc.tensor.matmul(yi[:, ys], bi, trs, start=False, stop=True)
                    # evac + store
                    ot = outpool.tile([128, 2, 512], _BF16, tag="out")
                    evac(ot[:, 0], yr[:], 512, allow=kp_allow)
                    evac(ot[:, 1], yi[:], 512, allow=kp_allow)
                    nc.sync.dma_start(out_d[sc, kp], ot[:])

            # A2 runs before B0: every weight/input DMA deadline gains the
            # width of one stage-A chunk, removing the early supply crunch
            stage_a(0)
            stage_a(1)
            stage_a(2)
            stage_b(0)
            stage_a(3)
            stage_b(1)

    nc.finalize()
    _nc_cache = nc
    return nc


# ---------------------------------------------------------------------------
# launch helper
# ---------------------------------------------------------------------------

_last_exec_ns = None


def last_exec_time_ns():
    """Sum of HW exec times (ns) of the launches in the last kernel() call,
    when KERNEL_TRACE=1 was set and NTFF profiling is available. None otherwise."""
    return _last_exec_ns


def predicted_exec_time_ns():
    """Cost-model (TimelineSim) predicted HW exec time for both launches, ns."""
    from concourse.timeline_sim import TimelineSim
    nc = _build_nc()
    return int(2 * TimelineSim(nc).simulate())


def _run_launch(cols_re, cols_im):
    """cols_re/cols_im: list of 8 planes [4096 f][512 b] f32.
    Returns list of 8 (Fre, Fim) planes [4096 k][512 b]."""
    global _last_exec_ns
    import os
    nc = _build_nc()
    bdc, bm = _make_consts()
    in_maps = []
    for c in range(NCORES):
        in_maps.append({
            "in2": _marshal_in(cols_re[c], cols_im[c]),
            "bdc": bdc, "bm": bm,
        })
    trace = bool(os.environ.get("KERNEL_TRACE"))
    try:
        res = run_bass_kernel_spmd(nc, in_maps, core_ids=list(range(NCORES)),
                                   trace=trace)
    except ModuleNotFoundError:
        # NTFF profiling hook unavailable under this axon client; run untraced.
        res = run_bass_kernel_spmd(nc, in_maps, core_ids=list(range(NCORES)))
    if trace and getattr(res, "exec_time_ns", None) is not None:
        _last_exec_ns = (_last_exec_ns or 0) + res.exec_time_ns
    return [_unmarshal_out(np.asarray(res.results[c]["out2"]))
            for c in range(NCORES)]


# ---------------------------------------------------------------------------
# public entry point
# ---------------------------------------------------------------------------

def kernel(x: np.ndarray) -> np.ndarray:
    """x: [N, 2] float32 (re, im). Returns FFT(x) as [N, 2] float32."""
    global _last_exec_ns
    _last_exec_ns = None
    x = np.asarray(x)
    Are = np.ascontiguousarray(x[:, 0].reshape(NG, NG))  # [j2g][j1g]
    Aim = np.ascontiguousarray(x[:, 1].reshape(NG, NG))

    # launch 1: FFT over rows (j2g) for each column j1g
    cols_re = [np.ascontiguousarray(Are[:, BPC * c:BPC * (c + 1)]) for c in range(NCORES)]
    cols_im = [np.ascontiguousarray(Aim[:, BPC * c:BPC * (c + 1)]) for c in range(NCORES)]
    l1 = _run_launch(cols_re, cols_im)

    # host: assemble F [k2g][j1g], twiddle, transpose-exchange
    F = np.empty((NG, NG), np.complex64)
    for c in range(NCORES):
        fre, fim = l1[c]
        F[:, BPC * c:BPC * (c + 1)] = fre + 1j * fim
    F *= _global_twiddle()

    # launch 2: FFT over j1g for each row k2g; core d gets rows [512d, 512(d+1))
    cols_re2 = []
    cols_im2 = []
    for d in range(NCORES):
        block = F[BPC * d:BPC * (d + 1), :].T      # [j1g][k2g-local]
        cols_re2.append(np.ascontiguousarray(block.real))
        cols_im2.append(np.ascontiguousarray(block.imag))
    l2 = _run_launch(cols_re2, cols_im2)

    # assemble Xmat [k1g][k2g]; out flat index k = 4096*k1g + k2g
    out = np.empty((NG, NG, 2), np.float32)
    for d in range(NCORES):
        rre, rim = l2[d]
        out[:, BPC * d:BPC * (d + 1), 0] = rre
        out[:, BPC * d:BPC * (d + 1), 1] = rim
    return out.reshape(N, 2)
